# revision 1
# baseline (speedup 1.0000x reference)
"""GPT-2-ish forward (B=4, T=1024, D=768, H=12, L=2, V=50257) on 8 trn2 cores.

Sharding: core pair (2b, 2b+1) both run the full trunk for batch b
(replicated, zero collectives); lm_head is vocab-split within the pair
(each core does 25600 of the host-padded 51200 vocab columns).

On-device layout: activations transposed [features, tokens]. Attention
uses attT [keys, q] as the stationary matmul operand with a ones-column
appended to V so the softmax denominator lands in the free dim of the
(att @ V_aug) output; normalization is then a tensor_scalar_mul.
LayerNorm stats via ones-vector matmuls (contraction over partitions);
(g*rstd) / (b - g*mean*rstd) broadcasts built as rank-1 matmuls in PSUM.
All matmuls bf16 with fp32 PSUM accumulation; residual stream fp32 in
SBUF; logits evicted fp16 and upcast on host.
"""

import numpy as np
import ml_dtypes
from contextlib import ExitStack

import concourse.bass as bass
from concourse import bacc
import concourse.mybir as mybir
import concourse.tile as tile
from concourse.bass_utils import run_bass_kernel_spmd
from concourse.masks import make_identity

BF16 = mybir.dt.bfloat16
F32 = mybir.dt.float32
F16 = mybir.dt.float16
AF = mybir.ActivationFunctionType
ALU = mybir.AluOpType

V = 50257
VPAD = 51200          # 2 * 25600
VSH = VPAD // 2       # per-core vocab shard
D = 768
H = 12
HD = 64
L = 2
T = 1024
B = 4
EPS = 1e-5
NKT = D // 128        # 6 k-tiles over D
NQC = T // 512        # 2 q-chunks
NTT = T // 128        # 8 token-tiles
NVC = VSH // 512      # 50 lm vocab chunks per core

TRACE = False
LAST_RESULT = None

_SINGLES = {}


def _ln_phase(tc, nc, tag, xt, g_d, b_d, out_tiles, small, scratch):
    """LayerNorm over partition dim (features) of xt (6 fp32 [128,1024] tiles).
    g_d/b_d: [768] bf16 DRAM APs. Writes bf16 out_tiles (6 x [128,1024])."""
    ones_bf = _SINGLES["ones_bf"]
    ones_row = _SINGLES["ones_row"]

    g_bf = small.tile([1, D], BF16, tag="g_bf", name="g_bf")
    b_bf = small.tile([1, D], BF16, tag="b_bf", name="b_bf")
    nc.sync.dma_start(g_bf, g_d.rearrange("(o d) -> o d", o=1))
    nc.sync.dma_start(b_bf, b_d.rearrange("(o d) -> o d", o=1))
    rstd_bf = small.tile([1, T], BF16, tag="rstd_bf", name="rstd_bf")
    nmr_bf = small.tile([1, T], BF16, tag="nmr_bf", name="nmr_bf")
    eps_sb = small.tile([1, 1], F32, tag="eps_sb", name="eps_sb")
    nc.vector.memset(eps_sb, EPS)

    with tc.tile_pool(name=f"stps_{tag}", bufs=1, space="PSUM") as stats_ps, \
         tc.tile_pool(name=f"abps_{tag}", bufs=2, space="PSUM") as ab_ps:
        for c in range(NQC):
            s1 = stats_ps.tile([1, 512], F32, tag="s1", name="s1")
            s2 = stats_ps.tile([1, 512], F32, tag="s2", name="s2")
            for kt in range(NKT):
                xbf = scratch.tile([128, 512], BF16, tag="xbf", name="xbf")
                sq = scratch.tile([128, 512], BF16, tag="sq", name="sq")
                xs = xt[kt][:, c * 512:(c + 1) * 512]
                nc.vector.tensor_copy(xbf, xs)
                nc.vector.tensor_mul(sq, xs, xs)
                nc.tensor.matmul(s1, ones_bf, xbf,
                                 start=(kt == 0), stop=(kt == NKT - 1))
                nc.tensor.matmul(s2, ones_bf, sq,
                                 start=(kt == 0), stop=(kt == NKT - 1))
            # mean = s1/D ; var = s2/D - mean^2 ; rstd = 1/sqrt(var+eps)
            mean = small.tile([1, 512], F32, tag="mean", name="mean")
            var = small.tile([1, 512], F32, tag="var", name="var")
            rstd = small.tile([1, 512], F32, tag="rstd", name="rstd")
            nc.vector.tensor_scalar_mul(mean, s1, 1.0 / D)
            nc.vector.tensor_mul(var, mean, mean)
            nc.vector.scalar_tensor_tensor(var, s2, 1.0 / D, var,
                                           op0=ALU.mult, op1=ALU.subtract)
            nc.scalar.activation(var, var, AF.Sqrt, bias=eps_sb)
            nc.vector.reciprocal(rstd, var)
            nc.vector.tensor_copy(rstd_bf[:, c * 512:(c + 1) * 512], rstd)
            # nmr = -mean*rstd
            nc.vector.scalar_tensor_tensor(var, mean, -1.0, rstd,
                                           op0=ALU.mult, op1=ALU.mult)
            nc.vector.tensor_copy(nmr_bf[:, c * 512:(c + 1) * 512], var)

        for kt in range(NKT):
            gs = g_bf[0:1, kt * 128:(kt + 1) * 128]
            bs = b_bf[0:1, kt * 128:(kt + 1) * 128]
            for c in range(NQC):
                cs = slice(c * 512, (c + 1) * 512)
                a_ps = ab_ps.tile([128, 512], F32, tag="a_ps", name="a_ps")
                b_ps = ab_ps.tile([128, 512], F32, tag="b_ps", name="b_ps")
                nc.tensor.matmul(a_ps, gs, rstd_bf[:, cs], start=True, stop=True)
                nc.tensor.matmul(b_ps, gs, nmr_bf[:, cs], start=True, stop=False)
                nc.tensor.matmul(b_ps, bs, ones_row[:, 0:512],
                                 start=False, stop=True)
                tmp = scratch.tile([128, 512], F32, tag="lntmp", name="lntmp")
                nc.vector.tensor_mul(tmp, xt[kt][:, cs], a_ps)
                nc.vector.tensor_add(out_tiles[kt][:, cs], tmp, b_ps)


def build_bass():
    nc = bacc.Bacc(None, target_bir_lowering=False)
    # ---- DRAM I/O (per-core shard views) ----
    xT_d = nc.dram_tensor("xT", [D, T], F32, kind="ExternalInput")
    qkw_d = nc.dram_tensor("qkw", [L, D, 2 * D], BF16, kind="ExternalInput")
    vw_d = nc.dram_tensor("vw", [L, D, D], BF16, kind="ExternalInput")
    pw_d = nc.dram_tensor("pw", [L, D, D], BF16, kind="ExternalInput")
    fcw_d = nc.dram_tensor("fcw", [L, D, 4 * D], BF16, kind="ExternalInput")
    fc2w_d = nc.dram_tensor("fc2w", [L, 4 * D, D], BF16, kind="ExternalInput")
    qkb_d = nc.dram_tensor("qkb", [L, 2 * D], F32, kind="ExternalInput")
    vb_d = nc.dram_tensor("vb", [L, D], BF16, kind="ExternalInput")
    pb_d = nc.dram_tensor("pb", [L, D], F32, kind="ExternalInput")
    fcb_d = nc.dram_tensor("fcb", [L, 4 * D], F32, kind="ExternalInput")
    fc2b_d = nc.dram_tensor("fc2b", [L, D], F32, kind="ExternalInput")
    ln_d = nc.dram_tensor("lnp", [L, 4, D], BF16, kind="ExternalInput")  # g1,b1,g2,b2
    lnf_d = nc.dram_tensor("lnf", [2, D], BF16, kind="ExternalInput")
    mask_d = nc.dram_tensor("mask", [4, 128, 512], BF16, kind="ExternalInput")
    lmw_d = nc.dram_tensor("lmw", [D, VSH], BF16, kind="ExternalInput")
    out_d = nc.dram_tensor("out", [T, VSH], F16, kind="ExternalOutput")

    with tile.TileContext(nc) as tc, ExitStack() as octx:
        singles = octx.enter_context(tc.tile_pool(name="singles", bufs=1))
        resid = octx.enter_context(tc.tile_pool(name="resid", bufs=1))

        # constants
        ones_bf = singles.tile([128, 1], BF16)
        nc.vector.memset(ones_bf, 1.0)
        ones_row = singles.tile([1, 512], BF16)
        nc.vector.memset(ones_row, 1.0)
        ident = singles.tile([128, 128], BF16)
        make_identity(nc, ident)
        _SINGLES["ones_bf"] = ones_bf
        _SINGLES["ones_row"] = ones_row

        mask_sb = singles.tile([128, 4, 512], BF16)
        nc.sync.dma_start(mask_sb, mask_d.rearrange("j p q -> p j q"))

        # residual stream, fp32, resident
        xt = [resid.tile([128, T], F32, tag=f"xt{i}", name=f"xt{i}") for i in range(NKT)]
        for kt in range(NKT):
            nc.sync.dma_start(xt[kt], xT_d[kt * 128:(kt + 1) * 128, :])

        for l in range(L):
            with ExitStack() as lctx:
                lnpool = lctx.enter_context(tc.tile_pool(name=f"ln{l}", bufs=1))
                wpool = lctx.enter_context(tc.tile_pool(name=f"w{l}", bufs=3))
                biasp = lctx.enter_context(tc.tile_pool(name=f"bias{l}", bufs=1))
                small = lctx.enter_context(tc.tile_pool(name=f"small{l}", bufs=2))
                scratch = lctx.enter_context(tc.tile_pool(name=f"scr{l}", bufs=3))

                qkb_sb = biasp.tile([128, 12], F32)
                nc.sync.dma_start(qkb_sb, qkb_d[l].rearrange("(t p) -> p t", p=128))
                vbbf_sb = biasp.tile([1, D], BF16)
                nc.sync.dma_start(vbbf_sb, vb_d[l].rearrange("(o d) -> o d", o=1))
                pb_sb = biasp.tile([128, 6], F32)
                nc.sync.dma_start(pb_sb, pb_d[l].rearrange("(t p) -> p t", p=128))
                fcb_sb = biasp.tile([128, 24], F32)
                nc.sync.dma_start(fcb_sb, fcb_d[l].rearrange("(t p) -> p t", p=128))
                fc2b_sb = biasp.tile([128, 6], F32)
                nc.sync.dma_start(fc2b_sb, fc2b_d[l].rearrange("(t p) -> p t", p=128))

                # ---------- LN1 ----------
                h_bf = [lnpool.tile([128, T], BF16, tag=f"hbf{i}", name=f"hbf{i}")
                        for i in range(NKT)]
                _ln_phase(tc, nc, f"l{l}a", xt, ln_d[l][0], ln_d[l][1],
                          h_bf, small, scratch)

                # ---------- qkT = (qk_w).T @ h  [1536, 1024] bf16 ----------
                qk_sb = [lnpool.tile([128, T], BF16, tag=f"qk{i}", name=f"qk{i}")
                         for i in range(12)]
                with tc.tile_pool(name=f"qkps{l}", bufs=3, space="PSUM") as qkps:
                    for f in range(12):
                        wt = wpool.tile([128, NKT, 128], BF16, tag="qkw_t", name="qkw_t")
                        nc.sync.dma_start(
                            wt, qkw_d[l][:, f * 128:(f + 1) * 128]
                            .rearrange("(t p) f -> p t f", p=128))
                        for c in range(NQC):
                            cs = slice(c * 512, (c + 1) * 512)
                            ps = qkps.tile([128, 512], F32, tag="qkps", name="qkps")
                            for kt in range(NKT):
                                nc.tensor.matmul(ps, wt[:, kt, :], h_bf[kt][:, cs],
                                                 start=(kt == 0),
                                                 stop=(kt == NKT - 1))
                            nc.scalar.activation(qk_sb[f][:, cs], ps, AF.Identity,
                                                 bias=qkb_sb[:, f:f + 1])

                    # ---------- V natural [tokens, 12, 65] bf16 (aug ones) ------
                    v_aug = [lnpool.tile([128, 12, 65], BF16, tag=f"vaug{i}", name=f"vaug{i}")
                             for i in range(NTT)]
                    vw_sb = [wpool.tile([128, D], BF16, tag=f"vw{i}", name=f"vw{i}", bufs=1)
                             for i in range(NKT)]
                    for kt in range(NKT):
                        nc.sync.dma_start(vw_sb[kt],
                                          vw_d[l][kt * 128:(kt + 1) * 128, :])
                    for tt in range(NTT):
                        nc.vector.memset(v_aug[tt][:, :, 64:65], 1.0)
                        for vc in range(2):
                            vs = slice(vc * 384, (vc + 1) * 384)
                            ps = qkps.tile([128, 384], F32, tag="vps", name="vps")
                            for kt in range(NKT):
                                nc.tensor.matmul(
                                    ps, h_bf[kt][:, tt * 128:(tt + 1) * 128],
                                    vw_sb[kt][:, vs],
                                    start=(kt == 0), stop=False)
                            nc.tensor.matmul(ps, ones_row[:, 0:128],
                                             vbbf_sb[:, vs],
                                             start=False, stop=True)
                            nc.vector.tensor_copy(
                                v_aug[tt][:, vc * 6:(vc + 1) * 6, 0:64],
                                ps.rearrange("p (h d) -> p h d", d=64))

                # ---------- attention per head-pair ----------
                attoT = [lnpool.tile([128, T], BF16, tag=f"attoT{i}", name=f"attoT{i}")
                         for i in range(NKT)]
                with tc.tile_pool(name=f"sps{l}", bufs=2, space="PSUM") as sps, \
                     tc.tile_pool(name=f"ops{l}", bufs=1, space="PSUM") as ops, \
                     tc.tile_pool(name=f"tps{l}", bufs=1, space="PSUM") as tps, \
                     tc.tile_pool(name=f"attp{l}", bufs=1) as attp:
                    for pr in range(6):
                        attT = [[attp.tile([128, T], BF16, tag=f"attT{hh}_{kt}", name=f"attT{hh}_{kt}")
                                 for kt in range(NTT)] for hh in range(2)]
                        psT = tps.tile([128, T], BF16, tag="psT", name="psT")
                        for c in range(NQC):
                            cs = slice(c * 512, (c + 1) * 512)
                            nkt = 4 * (c + 1)
                            for kt in range(nkt):
                                ks = slice(kt * 128, (kt + 1) * 128)
                                pss = [None, None]
                                for hh in range(2):
                                    ps = sps.tile([128, 512], F32, tag=f"sps{hh}", name=f"sps{hh}")
                                    pss[hh] = ps
                                    hs = slice(hh * 64, hh * 64 + 64)
                                    nc.tensor.matmul(
                                        ps,
                                        qk_sb[6 + pr][hs, ks],   # kT [64,128]
                                        qk_sb[pr][hs, cs],       # qT [64,512]
                                        start=True, stop=True)
                                partial = (c == 0) or (kt >= 4)
                                for hh in range(2):
                                    dst = attT[hh][kt][:, cs]
                                    nc.scalar.activation(dst, pss[hh], AF.Exp,
                                                         scale=0.125)
                                    if partial:
                                        nc.vector.tensor_mul(
                                            dst, dst, mask_sb[:, kt % 4, :])
                        for hh in range(2):
                            h = 2 * pr + hh
                            for qt in range(NTT):
                                po = ops.tile([128, 65], F32, tag=f"ops{hh}", name=f"ops{hh}")
                                for kt in range(qt + 1):
                                    nc.tensor.matmul(
                                        po,
                                        attT[hh][kt][:, qt * 128:(qt + 1) * 128],
                                        v_aug[kt][:, h, :],
                                        start=(kt == 0), stop=(kt == qt))
                                r_sb = scratch.tile([128, 1], F32, tag="r_sb", name="r_sb")
                                ao = scratch.tile([128, 64], BF16, tag="ao", name="ao")
                                nc.vector.reciprocal(r_sb, po[:, 64:65])
                                nc.vector.tensor_scalar_mul(ao, po[:, 0:64], r_sb)
                                nc.tensor.transpose(
                                    psT[hh * 64:hh * 64 + 64,
                                        qt * 128:(qt + 1) * 128],
                                    ao, ident,
                                    tile_position=(0, hh * 64))
                        nc.vector.tensor_copy(attoT[pr], psT)

                # ---------- proj + residual ----------
                pw_sb = [wpool.tile([128, D], BF16, tag=f"pw{i}", name=f"pw{i}", bufs=1)
                         for i in range(NKT)]
                for kt in range(NKT):
                    nc.sync.dma_start(pw_sb[kt], pw_d[l][kt * 128:(kt + 1) * 128, :])
                with tc.tile_pool(name=f"pps{l}", bufs=4, space="PSUM") as pps:
                    for ot in range(NKT):
                        for c in range(NQC):
                            cs = slice(c * 512, (c + 1) * 512)
                            ps = pps.tile([128, 512], F32, tag="pps", name="pps")
                            for kt in range(NKT):
                                nc.tensor.matmul(
                                    ps, pw_sb[kt][:, ot * 128:(ot + 1) * 128],
                                    attoT[kt][:, cs],
                                    start=(kt == 0), stop=(kt == NKT - 1))
                            nc.vector.scalar_tensor_tensor(
                                xt[ot][:, cs], ps, pb_sb[:, ot:ot + 1],
                                xt[ot][:, cs], op0=ALU.add, op1=ALU.add)

                # ---------- LN2 + MLP (token-chunked hidden) ----------
                h2in = [lnpool.tile([128, T], BF16, tag=f"hbf{i}", name=f"hbf{i}")
                        for i in range(NKT)]
                _ln_phase(tc, nc, f"l{l}b", xt, ln_d[l][2], ln_d[l][3],
                          h2in, small, scratch)

                with tc.tile_pool(name=f"mlpps{l}", bufs=3, space="PSUM") as mlpps, \
                     tc.tile_pool(name=f"h2p{l}", bufs=1) as h2p:
                    for c in range(NQC):
                        cs = slice(c * 512, (c + 1) * 512)
                        h2c = [h2p.tile([128, 512], BF16, tag=f"h2c{f}", name=f"h2c{f}")
                               for f in range(24)]
                        for f in range(24):
                            wt = wpool.tile([128, NKT, 128], BF16, tag="fcw_t", name="fcw_t")
                            nc.sync.dma_start(
                                wt, fcw_d[l][:, f * 128:(f + 1) * 128]
                                .rearrange("(t p) f -> p t f", p=128))
                            ps = mlpps.tile([128, 512], F32, tag="fcps", name="fcps")
                            for kt in range(NKT):
                                nc.tensor.matmul(ps, wt[:, kt, :], h2in[kt][:, cs],
                                                 start=(kt == 0),
                                                 stop=(kt == NKT - 1))
                            nc.scalar.activation(h2c[f], ps, AF.Gelu_apprx_tanh,
                                                 bias=fcb_sb[:, f:f + 1])
                        for ot in range(NKT):
                            wt = wpool.tile([128, 24, 128], BF16, tag="fc2w_t", name="fc2w_t", bufs=2)
                            nc.sync.dma_start(
                                wt, fc2w_d[l][:, ot * 128:(ot + 1) * 128]
                                .rearrange("(t p) f -> p t f", p=128))
                            ps = mlpps.tile([128, 512], F32, tag="fc2ps", name="fc2ps")
                            for kt in range(24):
                                nc.tensor.matmul(ps, wt[:, kt, :], h2c[kt],
                                                 start=(kt == 0), stop=(kt == 23))
                            nc.vector.scalar_tensor_tensor(
                                xt[ot][:, cs], ps, fc2b_sb[:, ot:ot + 1],
                                xt[ot][:, cs], op0=ALU.add, op1=ALU.add)

        # ---------- final LN + lm_head ----------
        with ExitStack() as fctx:
            lnpool = fctx.enter_context(tc.tile_pool(name="lnfp", bufs=1))
            biasp = fctx.enter_context(tc.tile_pool(name="biasf", bufs=1))
            small = fctx.enter_context(tc.tile_pool(name="smallf", bufs=2))
            scratch = fctx.enter_context(tc.tile_pool(name="scrf", bufs=3))
            xf_bf = [lnpool.tile([128, T], BF16, tag=f"xf{i}", name=f"xf{i}") for i in range(NKT)]
            _ln_phase(tc, nc, "lf", xt, lnf_d[0], lnf_d[1],
                      xf_bf, small, scratch)

            with tc.tile_pool(name="lmw", bufs=3) as lmwp, \
                 tc.tile_pool(name="lmps", bufs=4, space="PSUM") as lmps, \
                 tc.tile_pool(name="lmev", bufs=4) as lmev:
                for vc in range(NVC):
                    wt = lmwp.tile([128, NKT, 512], BF16, tag="lmw_t", name="lmw_t")
                    nc.sync.dma_start(
                        wt, lmw_d[:, vc * 512:(vc + 1) * 512]
                        .rearrange("(t p) v -> p t v", p=128))
                    for tt in range(NTT):
                        ps = lmps.tile([128, 512], F32, tag="lmps", name="lmps")
                        for kt in range(NKT):
                            nc.tensor.matmul(
                                ps, xf_bf[kt][:, tt * 128:(tt + 1) * 128],
                                wt[:, kt, :],
                                start=(kt == 0), stop=(kt == NKT - 1))
                        ev = lmev.tile([128, 512], F16, tag="lmev", name="lmev")
                        if tt % 2 == 0:
                            nc.scalar.copy(ev, ps)
                        else:
                            nc.vector.tensor_copy(ev, ps)
                        nc.sync.dma_start(
                            out_d[tt * 128:(tt + 1) * 128,
                                  vc * 512:(vc + 1) * 512], ev)
    nc.finalize()
    return nc


_NC_CACHE = None


def _get_nc():
    global _NC_CACHE
    if _NC_CACHE is None:
        _NC_CACHE = build_bass()
    return _NC_CACHE


def make_in_maps(idx, layer_num, wte, wpe, ln1_g, ln1_b, attn_w, attn_b, proj_w,
                 proj_b, ln2_g, ln2_b, fc_w, fc_b, fc2_w, fc2_b, lnf_g, lnf_b, lm_w):
    bf = ml_dtypes.bfloat16
    idx = np.asarray(idx)
    f32 = np.float32
    wte = np.asarray(wte, f32)
    wpe = np.asarray(wpe, f32)
    x0 = wte[idx] + wpe[:T]                      # [B,T,D] fp32 host embedding

    qkw = np.ascontiguousarray(np.asarray(attn_w, f32)[:, :, :2 * D]).astype(bf)
    vw = np.ascontiguousarray(np.asarray(attn_w, f32)[:, :, 2 * D:]).astype(bf)
    pw = np.asarray(proj_w, f32).astype(bf)
    fcw = np.asarray(fc_w, f32).astype(bf)
    fc2w = np.asarray(fc2_w, f32).astype(bf)
    qkb = np.ascontiguousarray(np.asarray(attn_b, f32)[:, :2 * D])
    vb = np.ascontiguousarray(np.asarray(attn_b, f32)[:, 2 * D:]).astype(bf)
    lnp = np.stack([np.asarray(ln1_g, f32), np.asarray(ln1_b, f32),
                    np.asarray(ln2_g, f32), np.asarray(ln2_b, f32)], axis=1).astype(bf)
    lnf = np.stack([np.asarray(lnf_g, f32), np.asarray(lnf_b, f32)], axis=0).astype(bf)

    lmw_pad = np.zeros((D, VPAD), f32)
    lmw_pad[:, :V] = np.asarray(lm_w, f32)
    lmw_bf = lmw_pad.astype(bf)

    # causal mask blocks: mask[j][kk, qq] = (128*j + kk) <= qq
    jj = np.arange(4)[:, None, None] * 128 + np.arange(128)[None, :, None]
    qq = np.arange(512)[None, None, :]
    mask = (jj <= qq).astype(bf)

    in_maps = []
    for core in range(8):
        b = core // 2
        vs = (core % 2) * VSH
        in_maps.append(dict(
            xT=np.ascontiguousarray(x0[b].T),
            qkw=qkw, vw=vw, pw=pw, fcw=fcw, fc2w=fc2w,
            qkb=qkb, vb=vb, pb=np.asarray(proj_b, f32),
            fcb=np.asarray(fc_b, f32), fc2b=np.asarray(fc2_b, f32),
            lnp=lnp, lnf=lnf, mask=mask,
            lmw=np.ascontiguousarray(lmw_bf[:, vs:vs + VSH]),
        ))
    return in_maps


def kernel(**inputs):
    global LAST_RESULT
    in_maps = make_in_maps(**inputs)
    nc = _get_nc()
    res = run_bass_kernel_spmd(nc, in_maps, core_ids=list(range(8)), trace=TRACE)
    LAST_RESULT = res

    logits = np.empty((B, T, V), np.float32)
    for b in range(B):
        lo = res.results[2 * b]["out"].astype(np.float32)
        hi = res.results[2 * b + 1]["out"].astype(np.float32)
        logits[b, :, :VSH] = lo
        logits[b, :, VSH:] = hi[:, :V - VSH]
    return logits



# revision 21
# speedup vs baseline: 1.0923x; 1.0923x over previous
"""GPT-2-ish forward (B=4, T=1024, D=768, H=12, L=2, V=50257) on 8 trn2 cores.

Sharding: core pair (2b, 2b+1) both run the full trunk for batch b
(replicated, zero collectives); lm_head is vocab-split within the pair
(each core does 25600 of the host-padded 51200 vocab columns).

On-device layout: activations transposed [features, tokens]. Attention
uses attT [keys, q] as the stationary matmul operand with a ones-column
appended to V so the softmax denominator lands in the free dim of the
(att @ V_aug) output; normalization is then a tensor_scalar_mul.

Perf notes (v2): the PE clock ramps 0.65 -> 1.2 -> 2.4 GHz with ~3us of
continuous execution needed for full speed, so the kernel is arranged to
keep the PE streaming: LN statistics use an all-ones [128,128] stationary
(sum broadcast to every PSUM partition) instead of rank-1 matmuls, with
the scale/shift applied on DVE/GPSIMD and the per-feature g/b applied via
scalar-engine activation (per-partition scale+bias). The attention loop
is software-pipelined in 12 (head-pair, half) units: the QK matmuls of
unit u+1 are issued before the att@V of unit u so the PE never waits on
the scalar-engine exp evictions. QK scores and exps only cover the
causal lower-triangle at 128-column granularity, with a single [128,128]
tril mask applied to diagonal blocks. The attn v-bias is folded into the
proj bias on the host. All matmuls bf16 with fp32 PSUM accumulation;
residual stream fp32 in SBUF; logits evicted fp16 and upcast on host.
"""

import numpy as np
import ml_dtypes
from contextlib import ExitStack

import concourse.bass as bass
from concourse import bacc
import concourse.mybir as mybir
import concourse.tile as tile
from concourse.bass_utils import run_bass_kernel_spmd
from concourse.masks import make_identity

BF16 = mybir.dt.bfloat16
F32 = mybir.dt.float32
F16 = mybir.dt.float16
AF = mybir.ActivationFunctionType
ALU = mybir.AluOpType

V = 50257
VPAD = 51200          # 2 * 25600
VSH = VPAD // 2       # per-core vocab shard
D = 768
H = 12
HD = 64
L = 2
T = 1024
B = 4
EPS = 1e-5
NKT = D // 128        # 6 k-tiles over D
NQC = T // 512        # 2 q-chunks
NTT = T // 128        # 8 token-tiles
NVC = VSH // 512      # 50 lm vocab chunks per core

TRACE = False
LAST_RESULT = None

_SINGLES = {}


def _ln_stats(tc, nc, xt, c, small, scratch, stats_ps):
    """Stats for chunk c: returns (mean_bc, rstd_bc) [128,512] fp32 SBUF,
    broadcast across partitions (identical rows)."""
    onesq = _SINGLES["onesq"]
    eps128 = _SINGLES["eps128"]
    cs = slice(c * 512, (c + 1) * 512)
    s1 = stats_ps.tile([128, 512], F32, tag="s1", name="s1")
    s2 = stats_ps.tile([128, 512], F32, tag="s2", name="s2")
    for kt in range(NKT):
        xbf = scratch.tile([128, 512], BF16, tag="xbf", name="xbf")
        sq = scratch.tile([128, 512], BF16, tag="sq", name="sq")
        xs = xt[kt][:, cs]
        nc.vector.tensor_copy(xbf, xs)
        nc.vector.tensor_mul(sq, xs, xs)
        nc.tensor.matmul(s1, onesq, xbf, start=(kt == 0), stop=(kt == NKT - 1))
        nc.tensor.matmul(s2, onesq, sq, start=(kt == 0), stop=(kt == NKT - 1))
    mean = small.tile([128, 512], F32, tag="mean", name="mean")
    rstd = small.tile([128, 512], F32, tag="rstd", name="rstd")
    var = scratch.tile([128, 512], F32, tag="var", name="var")
    nc.vector.tensor_scalar_mul(mean, s1, 1.0 / D)
    nc.vector.tensor_mul(var, mean, mean)
    nc.vector.scalar_tensor_tensor(var, s2, 1.0 / D, var,
                                   op0=ALU.mult, op1=ALU.subtract)
    nc.scalar.activation(var, var, AF.Sqrt, bias=eps128)
    nc.vector.reciprocal(rstd, var)
    return mean, rstd


def _ln_apply(tc, nc, xt, c, mean, rstd, g_sb, b_sb, out_tiles, scratch):
    """out[kt][:,cs] = (x - mean) * rstd * g[p] + b[p], bf16."""
    cs = slice(c * 512, (c + 1) * 512)
    for kt in range(NKT):
        t1 = scratch.tile([128, 512], F32, tag="lnt1", name="lnt1")
        eng = nc.gpsimd if kt % 2 == 0 else nc.vector
        eng.tensor_sub(t1, xt[kt][:, cs], mean)
        nc.vector.scalar_tensor_tensor(t1, t1, g_sb[:, kt:kt + 1], rstd,
                                       op0=ALU.mult, op1=ALU.mult)
        nc.scalar.activation(out_tiles[kt][:, cs], t1, AF.Identity,
                             bias=b_sb[:, kt:kt + 1])


def build_bass():
    nc = bacc.Bacc(None, target_bir_lowering=False)
    # ---- DRAM I/O (per-core shard views) ----
    xT_d = nc.dram_tensor("xT", [D, T], F32, kind="ExternalInput")
    qkw_d = nc.dram_tensor("qkw", [L, D, 2 * D], BF16, kind="ExternalInput")
    vw_d = nc.dram_tensor("vw", [L, D, D], BF16, kind="ExternalInput")
    pw_d = nc.dram_tensor("pw", [L, D, D], BF16, kind="ExternalInput")
    fcw_d = nc.dram_tensor("fcw", [L, D, 4 * D], BF16, kind="ExternalInput")
    fc2w_d = nc.dram_tensor("fc2w", [L, 4 * D, D], BF16, kind="ExternalInput")
    qkb_d = nc.dram_tensor("qkb", [L, 2 * D], F32, kind="ExternalInput")
    pb_d = nc.dram_tensor("pb", [L, D], F32, kind="ExternalInput")
    fcb_d = nc.dram_tensor("fcb", [L, 4 * D], F32, kind="ExternalInput")
    fc2b_d = nc.dram_tensor("fc2b", [L, D], F32, kind="ExternalInput")
    ln_d = nc.dram_tensor("lnp", [L, 4, D], F32, kind="ExternalInput")  # g1,b1,g2,b2
    lnf_d = nc.dram_tensor("lnf", [2, D], F32, kind="ExternalInput")
    tril_d = nc.dram_tensor("tril", [128, 128], BF16, kind="ExternalInput")
    lmw_d = nc.dram_tensor("lmw", [D, VSH], BF16, kind="ExternalInput")
    out_d = nc.dram_tensor("out", [T, VSH], F16, kind="ExternalOutput")

    with tile.TileContext(nc) as tc, ExitStack() as octx:
        singles = octx.enter_context(tc.tile_pool(name="singles", bufs=1))
        resid = octx.enter_context(tc.tile_pool(name="resid", bufs=1))

        # constants
        onesq = singles.tile([128, 128], BF16)
        nc.vector.memset(onesq, 1.0)
        eps128 = singles.tile([128, 1], F32)
        nc.vector.memset(eps128, EPS)
        ident = singles.tile([128, 128], BF16)
        make_identity(nc, ident)
        _SINGLES["onesq"] = onesq
        _SINGLES["eps128"] = eps128

        tril_sb = singles.tile([128, 128], BF16)
        nc.sync.dma_start(tril_sb, tril_d[:, :])

        # residual stream, fp32, resident
        xt = [resid.tile([128, T], F32, tag=f"xt{i}", name=f"xt{i}") for i in range(NKT)]
        for kt in range(NKT):
            nc.sync.dma_start(xt[kt], xT_d[kt * 128:(kt + 1) * 128, :])

        for l in range(L):
            with ExitStack() as lctx:
                lnpool = lctx.enter_context(tc.tile_pool(name=f"ln{l}", bufs=1))
                biasp = lctx.enter_context(tc.tile_pool(name=f"bias{l}", bufs=1))
                small = lctx.enter_context(tc.tile_pool(name=f"small{l}", bufs=2))
                scratch = lctx.enter_context(tc.tile_pool(name=f"scr{l}", bufs=3))
                # attention-scoped SBUF (weights + score/V tiles), freed at proj end
                actx = ExitStack()
                wpool = actx.enter_context(tc.tile_pool(name=f"w{l}", bufs=1))

                # ---- attention weight DMAs (early, one big tile each) ----
                qkw_sb = wpool.tile([128, NKT, 2 * D], BF16, name="qkw_sb")
                nc.sync.dma_start(qkw_sb,
                                  qkw_d[l].rearrange("(t p) f -> p t f", p=128))
                vw_sb = [wpool.tile([128, D], BF16, tag=f"vw{i}", name=f"vw{i}")
                         for i in range(NKT)]
                for kt in range(NKT):
                    nc.sync.dma_start(vw_sb[kt], vw_d[l][kt * 128:(kt + 1) * 128, :])
                pw_sb = [wpool.tile([128, D], BF16, tag=f"pw{i}", name=f"pw{i}")
                        for i in range(NKT)]
                for kt in range(NKT):
                    nc.sync.dma_start(pw_sb[kt], pw_d[l][kt * 128:(kt + 1) * 128, :])

                qkb_sb = biasp.tile([128, 12], F32)
                nc.sync.dma_start(qkb_sb, qkb_d[l].rearrange("(t p) -> p t", p=128))
                pb_sb = biasp.tile([128, 6], F32)
                nc.sync.dma_start(pb_sb, pb_d[l].rearrange("(t p) -> p t", p=128))
                fcb_sb = biasp.tile([128, 24], F32)
                nc.sync.dma_start(fcb_sb, fcb_d[l].rearrange("(t p) -> p t", p=128))
                fc2b_sb = biasp.tile([128, 6], F32)
                nc.sync.dma_start(fc2b_sb, fc2b_d[l].rearrange("(t p) -> p t", p=128))
                ln1g_sb = biasp.tile([128, 6], F32)
                nc.sync.dma_start(ln1g_sb, ln_d[l][0].rearrange("(t p) -> p t", p=128))
                ln1b_sb = biasp.tile([128, 6], F32)
                nc.sync.dma_start(ln1b_sb, ln_d[l][1].rearrange("(t p) -> p t", p=128))
                ln2g_sb = biasp.tile([128, 6], F32)
                nc.sync.dma_start(ln2g_sb, ln_d[l][2].rearrange("(t p) -> p t", p=128))
                ln2b_sb = biasp.tile([128, 6], F32)
                nc.sync.dma_start(ln2b_sb, ln_d[l][3].rearrange("(t p) -> p t", p=128))

                # ---------- LN1 + qkT, chunk-interleaved ----------
                h_bf = [lnpool.tile([128, T], BF16, tag=f"hbf{i}", name=f"hbf{i}")
                        for i in range(NKT)]
                qk_sb = [wpool.tile([128, T], BF16, tag=f"qk{i}", name=f"qk{i}")
                         for i in range(12)]

                def qkT_chunk(c, qkps):
                    cs = slice(c * 512, (c + 1) * 512)
                    for f in range(12):
                        ps = qkps.tile([128, 512], F32, tag="qkps", name="qkps")
                        for kt in range(NKT):
                            nc.tensor.matmul(
                                ps, qkw_sb[:, kt, f * 128:(f + 1) * 128],
                                h_bf[kt][:, cs],
                                start=(kt == 0), stop=(kt == NKT - 1))
                        nc.scalar.activation(qk_sb[f][:, cs], ps, AF.Identity,
                                             bias=qkb_sb[:, f:f + 1])

                with tc.tile_pool(name=f"stps{l}a", bufs=1, space="PSUM") as stats_ps, \
                     tc.tile_pool(name=f"qkps{l}", bufs=3, space="PSUM") as qkps:
                    m0, r0 = _ln_stats(tc, nc, xt, 0, small, scratch, stats_ps)
                    _ln_apply(tc, nc, xt, 0, m0, r0, ln1g_sb, ln1b_sb, h_bf, scratch)
                    m1, r1 = _ln_stats(tc, nc, xt, 1, small, scratch, stats_ps)
                    qkT_chunk(0, qkps)
                    _ln_apply(tc, nc, xt, 1, m1, r1, ln1g_sb, ln1b_sb, h_bf, scratch)
                    qkT_chunk(1, qkps)

                    # ---------- V natural [tokens, 12, 65] bf16 (aug ones) ----
                    v_aug = [wpool.tile([128, 12, 65], BF16, tag=f"vaug{i}", name=f"vaug{i}")
                             for i in range(NTT)]
                    for tt in range(NTT):
                        nc.vector.memset(v_aug[tt][:, :, 64:65], 1.0)
                        for vc in range(2):
                            vs = slice(vc * 384, (vc + 1) * 384)
                            ps = qkps.tile([128, 384], F32, tag="vps", name="vps")
                            for kt in range(NKT):
                                nc.tensor.matmul(
                                    ps, h_bf[kt][:, tt * 128:(tt + 1) * 128],
                                    vw_sb[kt][:, vs],
                                    start=(kt == 0), stop=(kt == NKT - 1))
                            nc.vector.tensor_copy(
                                v_aug[tt][:, vc * 6:(vc + 1) * 6, 0:64],
                                ps.rearrange("p (h d) -> p h d", d=64))

                # ---------- attention: 12 (pair, half) units, pipelined ----
                attoT = [lnpool.tile([128, T], BF16, tag=f"attoT{i}", name=f"attoT{i}")
                         for i in range(NKT)]
                with tc.tile_pool(name=f"sps{l}", bufs=4, space="PSUM") as sps, \
                     tc.tile_pool(name=f"ops{l}", bufs=2, space="PSUM") as ops, \
                     tc.tile_pool(name=f"tps{l}", bufs=2, space="PSUM") as tps, \
                     tc.tile_pool(name=f"attp{l}", bufs=2) as attp:

                    att_tiles = [None] * 12
                    psT_tiles = [None] * 6

                    def qk_unit(u):
                        pr, hh = u // 2, u % 2
                        hs = slice(hh * 64, hh * 64 + 64)
                        attT = [attp.tile([128, T], BF16, tag=f"attT{kt}",
                                          name=f"attT{kt}") for kt in range(NTT)]
                        att_tiles[u] = attT
                        for c in range(NQC):
                            for kt in range(4 * (c + 1)):
                                kb = kt - 4 * c
                                c0 = c * 512 + max(kb, 0) * 128
                                c1 = (c + 1) * 512
                                w = c1 - c0
                                ps = sps.tile([128, 512], F32, tag="sps", name="sps")
                                nc.tensor.matmul(
                                    ps[:, 0:w],
                                    qk_sb[6 + pr][hs, kt * 128:(kt + 1) * 128],
                                    qk_sb[pr][hs, c0:c1],
                                    start=True, stop=True)
                                nc.scalar.activation(attT[kt][:, c0:c1], ps[:, 0:w],
                                                     AF.Exp, scale=0.125)
                                if kb >= 0:
                                    nc.gpsimd.tensor_mul(
                                        attT[kt][:, c0:c0 + 128],
                                        attT[kt][:, c0:c0 + 128], tril_sb)

                    def av_unit(u):
                        pr, hh = u // 2, u % 2
                        h = 2 * pr + hh
                        attT = att_tiles[u]
                        if hh == 0:
                            psT_tiles[pr] = tps.tile([128, T], BF16, tag="psT",
                                                     name="psT")
                        psT = psT_tiles[pr]
                        for qt in range(NTT):
                            po = ops.tile([128, 65], F32, tag="po", name="po")
                            for kt in range(qt + 1):
                                nc.tensor.matmul(
                                    po, attT[kt][:, qt * 128:(qt + 1) * 128],
                                    v_aug[kt][:, h, :],
                                    start=(kt == 0), stop=(kt == qt))
                            r_sb = scratch.tile([128, 1], F32, tag="r_sb", name="r_sb")
                            ao = scratch.tile([128, 64], BF16, tag="ao", name="ao")
                            nc.vector.reciprocal(r_sb, po[:, 64:65])
                            if qt % 2 == 0:
                                nc.vector.tensor_scalar_mul(ao, po[:, 0:64], r_sb)
                            else:
                                nc.scalar.activation(ao, po[:, 0:64], AF.Identity,
                                                     scale=r_sb)
                            nc.tensor.transpose(
                                psT[hh * 64:hh * 64 + 64,
                                    qt * 128:(qt + 1) * 128],
                                ao, ident,
                                tile_position=(0, hh * 64))
                        if hh == 1:
                            if pr % 2 == 0:
                                nc.vector.tensor_copy(attoT[pr], psT)
                            else:
                                nc.scalar.copy(attoT[pr], psT)

                    qk_unit(0)
                    for u in range(12):
                        if u + 1 < 12:
                            qk_unit(u + 1)
                        av_unit(u)

                # ---------- proj + residual ----------
                with tc.tile_pool(name=f"pps{l}", bufs=4, space="PSUM") as pps:
                    for ot in range(NKT):
                        for c in range(NQC):
                            cs = slice(c * 512, (c + 1) * 512)
                            ps = pps.tile([128, 512], F32, tag="pps", name="pps")
                            for kt in range(NKT):
                                nc.tensor.matmul(
                                    ps, pw_sb[kt][:, ot * 128:(ot + 1) * 128],
                                    attoT[kt][:, cs],
                                    start=(kt == 0), stop=(kt == NKT - 1))
                            if (ot + c) % 2 == 0:
                                nc.vector.scalar_tensor_tensor(
                                    xt[ot][:, cs], ps, pb_sb[:, ot:ot + 1],
                                    xt[ot][:, cs], op0=ALU.add, op1=ALU.add)
                            else:
                                tmp = scratch.tile([128, 512], F32, tag="rtmp",
                                                   name="rtmp")
                                nc.scalar.activation(tmp, ps, AF.Identity,
                                                     bias=pb_sb[:, ot:ot + 1])
                                nc.gpsimd.tensor_add(xt[ot][:, cs],
                                                     xt[ot][:, cs], tmp)
                actx.close()

                # ---------- LN2 + MLP (chunk-interleaved) ----------
                h2in = [lnpool.tile([128, T], BF16, tag=f"hbf{i}", name=f"hbf{i}")
                        for i in range(NKT)]

                def mlp_chunk(c, mlpps, h2p, fcw_sb, fc2w_sb):
                    cs = slice(c * 512, (c + 1) * 512)
                    h2c = [h2p.tile([128, 512], BF16, tag=f"h2c{f}", name=f"h2c{f}")
                           for f in range(24)]
                    for f in range(24):
                        ps = mlpps.tile([128, 512], F32, tag="fcps", name="fcps")
                        for kt in range(NKT):
                            nc.tensor.matmul(
                                ps, fcw_sb[:, kt, f * 128:(f + 1) * 128],
                                h2in[kt][:, cs],
                                start=(kt == 0), stop=(kt == NKT - 1))
                        nc.scalar.activation(h2c[f], ps, AF.Gelu_apprx_tanh,
                                             bias=fcb_sb[:, f:f + 1])
                    for ot in range(NKT):
                        ps = mlpps.tile([128, 512], F32, tag="fc2ps", name="fc2ps")
                        for kt in range(24):
                            nc.tensor.matmul(ps, fc2w_sb[:, kt, ot * 128:(ot + 1) * 128],
                                             h2c[kt],
                                             start=(kt == 0), stop=(kt == 23))
                        if ot % 2 == 0:
                            nc.vector.scalar_tensor_tensor(
                                xt[ot][:, cs], ps, fc2b_sb[:, ot:ot + 1],
                                xt[ot][:, cs], op0=ALU.add, op1=ALU.add)
                        else:
                            tmp = scratch.tile([128, 512], F32, tag="rtmp",
                                               name="rtmp")
                            nc.scalar.activation(tmp, ps, AF.Identity,
                                                 bias=fc2b_sb[:, ot:ot + 1])
                            nc.gpsimd.tensor_add(xt[ot][:, cs],
                                                 xt[ot][:, cs], tmp)

                with tc.tile_pool(name=f"stps{l}b", bufs=1, space="PSUM") as stats_ps, \
                     tc.tile_pool(name=f"mlpps{l}", bufs=3, space="PSUM") as mlpps, \
                     tc.tile_pool(name=f"mlpw{l}", bufs=1) as mlpw, \
                     tc.tile_pool(name=f"h2p{l}", bufs=1) as h2p:
                    fcw_sb = mlpw.tile([128, NKT, 4 * D], BF16, name="fcw_sb")
                    nc.sync.dma_start(fcw_sb,
                                      fcw_d[l].rearrange("(t p) f -> p t f", p=128))
                    fc2w_sb = mlpw.tile([128, 24, D], BF16, name="fc2w_sb")
                    nc.sync.dma_start(fc2w_sb,
                                      fc2w_d[l].rearrange("(t p) f -> p t f", p=128))
                    m0, r0 = _ln_stats(tc, nc, xt, 0, small, scratch, stats_ps)
                    _ln_apply(tc, nc, xt, 0, m0, r0, ln2g_sb, ln2b_sb, h2in, scratch)
                    m1, r1 = _ln_stats(tc, nc, xt, 1, small, scratch, stats_ps)
                    mlp_chunk(0, mlpps, h2p, fcw_sb, fc2w_sb)
                    _ln_apply(tc, nc, xt, 1, m1, r1, ln2g_sb, ln2b_sb, h2in, scratch)
                    mlp_chunk(1, mlpps, h2p, fcw_sb, fc2w_sb)

        # ---------- final LN + lm_head ----------
        with ExitStack() as fctx:
            lnpool = fctx.enter_context(tc.tile_pool(name="lnfp", bufs=1))
            biasp = fctx.enter_context(tc.tile_pool(name="biasf", bufs=1))
            small = fctx.enter_context(tc.tile_pool(name="smallf", bufs=2))
            scratch = fctx.enter_context(tc.tile_pool(name="scrf", bufs=3))
            lmwp = fctx.enter_context(tc.tile_pool(name="lmw", bufs=4))

            lnfg_sb = biasp.tile([128, 6], F32)
            nc.sync.dma_start(lnfg_sb, lnf_d[0].rearrange("(t p) -> p t", p=128))
            lnfb_sb = biasp.tile([128, 6], F32)
            nc.sync.dma_start(lnfb_sb, lnf_d[1].rearrange("(t p) -> p t", p=128))

            # prefetch first lm weight tiles while final LN runs
            lm_wt = {}
            def lm_fetch(vc):
                wt = lmwp.tile([128, NKT, 512], BF16, tag="lmw_t", name="lmw_t")
                nc.sync.dma_start(
                    wt, lmw_d[:, vc * 512:(vc + 1) * 512]
                    .rearrange("(t p) v -> p t v", p=128))
                lm_wt[vc] = wt
            lm_fetch(0)
            lm_fetch(1)

            xf_bf = [lnpool.tile([128, T], BF16, tag=f"xf{i}", name=f"xf{i}")
                     for i in range(NKT)]
            with tc.tile_pool(name="stpsf", bufs=1, space="PSUM") as stats_ps:
                m0, r0 = _ln_stats(tc, nc, xt, 0, small, scratch, stats_ps)
                _ln_apply(tc, nc, xt, 0, m0, r0, lnfg_sb, lnfb_sb, xf_bf, scratch)
                m1, r1 = _ln_stats(tc, nc, xt, 1, small, scratch, stats_ps)
                _ln_apply(tc, nc, xt, 1, m1, r1, lnfg_sb, lnfb_sb, xf_bf, scratch)

            with tc.tile_pool(name="lmps", bufs=4, space="PSUM") as lmps, \
                 tc.tile_pool(name="lmev", bufs=6) as lmev:
                for vc in range(NVC):
                    if vc + 2 < NVC:
                        lm_fetch(vc + 2)
                    wt = lm_wt.pop(vc)
                    for tt in range(NTT):
                        ps = lmps.tile([128, 512], F32, tag="lmps", name="lmps")
                        for kt in range(NKT):
                            nc.tensor.matmul(
                                ps, xf_bf[kt][:, tt * 128:(tt + 1) * 128],
                                wt[:, kt, :],
                                start=(kt == 0), stop=(kt == NKT - 1))
                        ev = lmev.tile([128, 512], F16, tag="lmev", name="lmev")
                        if tt % 2 == 0:
                            nc.scalar.copy(ev, ps)
                        else:
                            nc.vector.tensor_copy(ev, ps)
                        nc.sync.dma_start(
                            out_d[tt * 128:(tt + 1) * 128,
                                  vc * 512:(vc + 1) * 512], ev)
    nc.finalize()
    return nc


_NC_CACHE = None


def _get_nc():
    global _NC_CACHE
    if _NC_CACHE is None:
        _NC_CACHE = build_bass()
    return _NC_CACHE


def make_in_maps(idx, layer_num, wte, wpe, ln1_g, ln1_b, attn_w, attn_b, proj_w,
                 proj_b, ln2_g, ln2_b, fc_w, fc_b, fc2_w, fc2_b, lnf_g, lnf_b, lm_w):
    bf = ml_dtypes.bfloat16
    idx = np.asarray(idx)
    f32 = np.float32
    wte = np.asarray(wte, f32)
    wpe = np.asarray(wpe, f32)
    x0 = wte[idx] + wpe[:T]                      # [B,T,D] fp32 host embedding

    attn_w = np.asarray(attn_w, f32)
    attn_b = np.asarray(attn_b, f32)
    proj_w = np.asarray(proj_w, f32)
    qkw = np.ascontiguousarray(attn_w[:, :, :2 * D]).astype(bf)
    vw = np.ascontiguousarray(attn_w[:, :, 2 * D:]).astype(bf)
    pw = proj_w.astype(bf)
    fcw = np.asarray(fc_w, f32).astype(bf)
    fc2w = np.asarray(fc2_w, f32).astype(bf)
    qkb = np.ascontiguousarray(attn_b[:, :2 * D])
    vb = np.ascontiguousarray(attn_b[:, 2 * D:])            # [L, D]
    # v-bias folds into the proj bias: y = att@(h@vw + vb) = att@(h@vw) + vb
    pb_fold = np.einsum('ld,lde->le', vb, proj_w) + np.asarray(proj_b, f32)
    lnp = np.stack([np.asarray(ln1_g, f32), np.asarray(ln1_b, f32),
                    np.asarray(ln2_g, f32), np.asarray(ln2_b, f32)], axis=1)
    lnf = np.stack([np.asarray(lnf_g, f32), np.asarray(lnf_b, f32)], axis=0)

    lmw_pad = np.zeros((D, VPAD), f32)
    lmw_pad[:, :V] = np.asarray(lm_w, f32)
    lmw_bf = lmw_pad.astype(bf)

    # tril mask for diagonal 128x128 blocks: tril[kk, qq] = kk <= qq
    tril = (np.arange(128)[:, None] <= np.arange(128)[None, :]).astype(bf)

    in_maps = []
    for core in range(8):
        b = core // 2
        vs = (core % 2) * VSH
        in_maps.append(dict(
            xT=np.ascontiguousarray(x0[b].T),
            qkw=qkw, vw=vw, pw=pw, fcw=fcw, fc2w=fc2w,
            qkb=qkb, pb=pb_fold,
            fcb=np.asarray(fc_b, f32), fc2b=np.asarray(fc2_b, f32),
            lnp=lnp, lnf=lnf, tril=tril,
            lmw=np.ascontiguousarray(lmw_bf[:, vs:vs + VSH]),
        ))
    return in_maps


def kernel(**inputs):
    global LAST_RESULT
    in_maps = make_in_maps(**inputs)
    nc = _get_nc()
    res = run_bass_kernel_spmd(nc, in_maps, core_ids=list(range(8)), trace=TRACE)
    LAST_RESULT = res

    logits = np.empty((B, T, V), np.float32)
    for b in range(B):
        lo = res.results[2 * b]["out"].astype(np.float32)
        hi = res.results[2 * b + 1]["out"].astype(np.float32)
        logits[b, :, :VSH] = lo
        logits[b, :, VSH:] = hi[:, :V - VSH]
    return logits


# revision 22
# speedup vs baseline: 1.1551x; 1.0575x over previous
"""GPT-2-ish forward (B=4, T=1024, D=768, H=12, L=2, V=50257) on 8 trn2 cores.

Sharding (v3): core pair (2b, 2b+1) sequence-splits the trunk for batch b.
Side s = core%2 owns the even (s=0) or odd (s=1) 128-token tiles of the
1024-token sequence: tiles {s, s+2, s+4, s+6} (interleaving balances the
causal attention load: key-tile needs sum to 16 vs 20). Each core runs
LN / qkv / proj / MLP on its own 512 tokens; K and V are pair-AllGathered
per layer (DRAM bounce) so attention sees all 1024 keys; the final-LN
output is pair-AllGathered before a vocab-split lm_head identical to v2
(each core: all 1024 tokens x 25600 vocab columns).

The SPMD program is identical on every core. Side-dependent causal
masking is data: slot j's last two key tiles (2j, 2j+1) are multiplied by
a per-core [128, 2, 128] mask md = (tril, zeros) on side 0 and
(ones, tril) on side 1. QK scores/exps only cover key tile kt from query
slot kt//2 onward; exp activations are issued per kt-pair on a
[128, 2, 512] PSUM tile to halve the activation-instruction count.

Engine layout as v2: PE ramps to 2.4 GHz only when streaming, so QK
matmuls of attention unit u+1 are issued before att@V of unit u; PSUM
evictions alternate DVE/scalar; GPSIMD (Pool, SBUF-only) takes masks,
LN subtract and residual adds. All matmuls bf16 with fp32 PSUM
accumulation; residual fp32; logits f16, upcast on host.
"""

import numpy as np
import ml_dtypes
from contextlib import ExitStack

import concourse.bass as bass
from concourse import bacc
import concourse.mybir as mybir
import concourse.tile as tile
from concourse.bass_utils import run_bass_kernel_spmd
from concourse.masks import make_identity

BF16 = mybir.dt.bfloat16
F32 = mybir.dt.float32
F16 = mybir.dt.float16
AF = mybir.ActivationFunctionType
ALU = mybir.AluOpType

V = 50257
VPAD = 51200          # 2 * 25600
VSH = VPAD // 2       # per-core vocab shard
D = 768
H = 12
HD = 64
L = 2
T = 1024
TC = 512              # tokens per core (trunk)
B = 4
EPS = 1e-5
NKT = D // 128        # 6 k-tiles over D
NTT = T // 128        # 8 global token-tiles
NSL = TC // 128       # 4 local token slots
NVC = VSH // 512      # 50 lm vocab chunks per core
GROUPS = [[0, 1], [2, 3], [4, 5], [6, 7]]

TRACE = False
LAST_RESULT = None

_SINGLES = {}


def _ln_stats(tc, nc, xt, small, scratch, stats_ps):
    """Stats over the core's 512 tokens: (mean, rstd) [128,512] fp32,
    broadcast across partitions."""
    onesq = _SINGLES["onesq"]
    eps128 = _SINGLES["eps128"]
    s1 = stats_ps.tile([128, TC], F32, tag="s1", name="s1")
    s2 = stats_ps.tile([128, TC], F32, tag="s2", name="s2")
    for kt in range(NKT):
        xbf = scratch.tile([128, TC], BF16, tag="xbf", name="xbf")
        sq = scratch.tile([128, TC], BF16, tag="sq", name="sq")
        nc.vector.tensor_copy(xbf, xt[kt])
        nc.vector.tensor_mul(sq, xt[kt], xt[kt])
        nc.tensor.matmul(s1, onesq, xbf, start=(kt == 0), stop=(kt == NKT - 1))
        nc.tensor.matmul(s2, onesq, sq, start=(kt == 0), stop=(kt == NKT - 1))
    mean = small.tile([128, TC], F32, tag="mean", name="mean")
    rstd = small.tile([128, TC], F32, tag="rstd", name="rstd")
    var = scratch.tile([128, TC], F32, tag="var", name="var")
    nc.vector.tensor_scalar_mul(mean, s1, 1.0 / D)
    nc.vector.tensor_mul(var, mean, mean)
    nc.vector.scalar_tensor_tensor(var, s2, 1.0 / D, var,
                                   op0=ALU.mult, op1=ALU.subtract)
    nc.scalar.activation(var, var, AF.Sqrt, bias=eps128)
    nc.vector.reciprocal(rstd, var)
    return mean, rstd


def _ln_apply(tc, nc, xt, mean, rstd, g_sb, b_sb, out_tiles, scratch):
    for kt in range(NKT):
        t1 = scratch.tile([128, TC], F32, tag="lnt1", name="lnt1")
        eng = nc.gpsimd if kt % 2 == 0 else nc.vector
        eng.tensor_sub(t1, xt[kt], mean)
        nc.vector.scalar_tensor_tensor(t1, t1, g_sb[:, kt:kt + 1], rstd,
                                       op0=ALU.mult, op1=ALU.mult)
        nc.scalar.activation(out_tiles[kt], t1, AF.Identity,
                             bias=b_sb[:, kt:kt + 1])


def build_bass():
    nc = bacc.Bacc(None, target_bir_lowering=False)
    # ---- DRAM I/O (per-core shard views) ----
    xT_d = nc.dram_tensor("xT", [D, TC], F32, kind="ExternalInput")
    qkw_d = nc.dram_tensor("qkw", [L, D, 2 * D], BF16, kind="ExternalInput")
    vw_d = nc.dram_tensor("vw", [L, D, D], BF16, kind="ExternalInput")
    pw_d = nc.dram_tensor("pw", [L, D, D], BF16, kind="ExternalInput")
    fcw_d = nc.dram_tensor("fcw", [L, D, 4 * D], BF16, kind="ExternalInput")
    fc2w_d = nc.dram_tensor("fc2w", [L, 4 * D, D], BF16, kind="ExternalInput")
    qkb_d = nc.dram_tensor("qkb", [L, 2 * D], F32, kind="ExternalInput")
    pb_d = nc.dram_tensor("pb", [L, D], F32, kind="ExternalInput")
    fcb_d = nc.dram_tensor("fcb", [L, 4 * D], F32, kind="ExternalInput")
    fc2b_d = nc.dram_tensor("fc2b", [L, D], F32, kind="ExternalInput")
    ln_d = nc.dram_tensor("lnp", [L, 4, D], F32, kind="ExternalInput")
    lnf_d = nc.dram_tensor("lnf", [2, D], F32, kind="ExternalInput")
    md_d = nc.dram_tensor("md", [128, 2, 128], BF16, kind="ExternalInput")
    lmw_d = nc.dram_tensor("lmw", [D, VSH], BF16, kind="ExternalInput")
    out_d = nc.dram_tensor("out", [T, VSH], F16, kind="ExternalOutput")

    with tile.TileContext(nc) as tc, ExitStack() as octx:
        singles = octx.enter_context(tc.tile_pool(name="singles", bufs=1))
        resid = octx.enter_context(tc.tile_pool(name="resid", bufs=1))
        dram = octx.enter_context(tc.tile_pool(name="dram", bufs=1, space="DRAM"))

        onesq = singles.tile([128, 128], BF16)
        nc.vector.memset(onesq, 1.0)
        eps128 = singles.tile([128, 1], F32)
        nc.vector.memset(eps128, EPS)
        ident = singles.tile([128, 128], BF16)
        make_identity(nc, ident)
        _SINGLES["onesq"] = onesq
        _SINGLES["eps128"] = eps128

        md_sb = singles.tile([128, 2, 128], BF16)
        nc.sync.dma_start(md_sb, md_d[:, :, :])

        xt = [resid.tile([128, TC], F32, tag=f"xt{i}", name=f"xt{i}")
              for i in range(NKT)]
        for kt in range(NKT):
            nc.sync.dma_start(xt[kt], xT_d[kt * 128:(kt + 1) * 128, :])

        for l in range(L):
            with ExitStack() as lctx:
                lnpool = lctx.enter_context(tc.tile_pool(name=f"ln{l}", bufs=1))
                biasp = lctx.enter_context(tc.tile_pool(name=f"bias{l}", bufs=1))
                small = lctx.enter_context(tc.tile_pool(name=f"small{l}", bufs=2))
                scratch = lctx.enter_context(tc.tile_pool(name=f"scr{l}", bufs=3))
                actx = ExitStack()
                wpool = actx.enter_context(tc.tile_pool(name=f"w{l}", bufs=1))

                qkw_sb = wpool.tile([128, NKT, 2 * D], BF16, name="qkw_sb")
                nc.sync.dma_start(qkw_sb,
                                  qkw_d[l].rearrange("(t p) f -> p t f", p=128))
                vw_sb = [wpool.tile([128, D], BF16, tag=f"vw{i}", name=f"vw{i}")
                         for i in range(NKT)]
                for kt in range(NKT):
                    nc.sync.dma_start(vw_sb[kt], vw_d[l][kt * 128:(kt + 1) * 128, :])
                pw_sb = [wpool.tile([128, D], BF16, tag=f"pw{i}", name=f"pw{i}")
                        for i in range(NKT)]
                for kt in range(NKT):
                    nc.sync.dma_start(pw_sb[kt], pw_d[l][kt * 128:(kt + 1) * 128, :])

                qkb_sb = biasp.tile([128, 12], F32)
                nc.sync.dma_start(qkb_sb, qkb_d[l].rearrange("(t p) -> p t", p=128))
                pb_sb = biasp.tile([128, 6], F32)
                nc.sync.dma_start(pb_sb, pb_d[l].rearrange("(t p) -> p t", p=128))
                fcb_sb = biasp.tile([128, 24], F32)
                nc.sync.dma_start(fcb_sb, fcb_d[l].rearrange("(t p) -> p t", p=128))
                fc2b_sb = biasp.tile([128, 6], F32)
                nc.sync.dma_start(fc2b_sb, fc2b_d[l].rearrange("(t p) -> p t", p=128))
                ln_sb = []
                for i in range(4):
                    t = biasp.tile([128, 6], F32, tag=f"lnp{i}", name=f"lnp{i}")
                    nc.sync.dma_start(t, ln_d[l][i].rearrange("(t p) -> p t", p=128))
                    ln_sb.append(t)

                # ---------- LN1 ----------
                h_bf = [lnpool.tile([128, TC], BF16, tag=f"hbf{i}", name=f"hbf{i}")
                        for i in range(NKT)]
                qk_sb = [wpool.tile([128, TC], BF16, tag=f"qk{i}", name=f"qk{i}")
                         for i in range(12)]

                with tc.tile_pool(name=f"stps{l}a", bufs=1, space="PSUM") as stats_ps, \
                     tc.tile_pool(name=f"qkps{l}", bufs=3, space="PSUM") as qkps:
                    m0, r0 = _ln_stats(tc, nc, xt, small, scratch, stats_ps)
                    _ln_apply(tc, nc, xt, m0, r0, ln_sb[0], ln_sb[1], h_bf, scratch)

                    # K features first (f 6..11) so the K gather starts early
                    for f in list(range(6, 12)) + list(range(6)):
                        ps = qkps.tile([128, TC], F32, tag="qkps", name="qkps")
                        for kt in range(NKT):
                            nc.tensor.matmul(
                                ps, qkw_sb[:, kt, f * 128:(f + 1) * 128],
                                h_bf[kt],
                                start=(kt == 0), stop=(kt == NKT - 1))
                        nc.scalar.activation(qk_sb[f], ps, AF.Identity,
                                             bias=qkb_sb[:, f:f + 1])
                        if f == 11:
                            # ---- K AllGather (pair) ----
                            kb_in = dram.tile([6, 128, TC], BF16, tag=f"kbi{l}",
                                              name=f"kbi{l}")
                            kb_out = dram.tile([2, 6, 128, TC], BF16, tag=f"kbo{l}",
                                               name=f"kbo{l}")
                            for i in range(6):
                                nc.gpsimd.dma_start(kb_in[i], qk_sb[6 + i])
                            nc.gpsimd.collective_compute(
                                "AllGather", mybir.AluOpType.bypass,
                                replica_groups=GROUPS,
                                ins=[kb_in[:].opt()], outs=[kb_out[:].opt()])

                    # ---------- V own tiles [128, 12, 65] (incl ones col) ----
                    v_own = [wpool.tile([128, 12, 65], BF16, tag=f"vown{i}",
                                        name=f"vown{i}") for i in range(NSL)]
                    for tt in range(NSL):
                        nc.vector.memset(v_own[tt][:, :, 64:65], 1.0)
                        for vc in range(2):
                            vs = slice(vc * 384, (vc + 1) * 384)
                            ps = qkps.tile([128, 384], F32, tag="vps", name="vps")
                            for kt in range(NKT):
                                nc.tensor.matmul(
                                    ps, h_bf[kt][:, tt * 128:(tt + 1) * 128],
                                    vw_sb[kt][:, vs],
                                    start=(kt == 0), stop=(kt == NKT - 1))
                            nc.vector.tensor_copy(
                                v_own[tt][:, vc * 6:(vc + 1) * 6, 0:64],
                                ps.rearrange("p (h d) -> p h d", d=64))

                    # ---- V AllGather (pair) ----
                    vb_in = dram.tile([NSL, 128, 12 * 65], BF16, tag=f"vbi{l}",
                                      name=f"vbi{l}")
                    vb_out = dram.tile([2, NSL, 128, 12 * 65], BF16, tag=f"vbo{l}",
                                       name=f"vbo{l}")
                    for i in range(NSL):
                        nc.gpsimd.dma_start(
                            vb_in[i], v_own[i].rearrange("p h d -> p (h d)"))
                    nc.gpsimd.collective_compute(
                        "AllGather", mybir.AluOpType.bypass,
                        replica_groups=GROUPS,
                        ins=[vb_in[:].opt()], outs=[vb_out[:].opt()])

                    # ---- gathered K back to SBUF: kT_all[f] [128, 1024] ----
                    kT_all = [wpool.tile([128, T], BF16, tag=f"kta{i}",
                                         name=f"kta{i}") for i in range(6)]
                    for f in range(6):
                        for s in range(2):
                            dst = kT_all[f].rearrange(
                                "p (t two c) -> p t two c", two=2, c=128)[:, :, s, :]
                            nc.sync.dma_start(
                                dst, kb_out[s, f].rearrange(
                                    "p (t c) -> p t c", c=128))
                    # ---- gathered V back: v_all[g] [128, 12, 65] ----
                    v_all = [wpool.tile([128, 12, 65], BF16, tag=f"vall{i}",
                                        name=f"vall{i}") for i in range(NTT)]
                    for g in range(NTT):
                        nc.sync.dma_start(
                            v_all[g],
                            vb_out[g % 2, g // 2].rearrange("p (h d) -> p h d", d=65))

                # ---------- attention: 12 (pair, half) units, pipelined ----
                attoT = [lnpool.tile([128, TC], BF16, tag=f"attoT{i}", name=f"attoT{i}")
                         for i in range(NKT)]
                with tc.tile_pool(name=f"sps{l}", bufs=2, space="PSUM") as sps, \
                     tc.tile_pool(name=f"ops{l}", bufs=2, space="PSUM") as ops, \
                     tc.tile_pool(name=f"tps{l}", bufs=2, space="PSUM") as tps, \
                     tc.tile_pool(name=f"attp{l}", bufs=2) as attp:

                    att_tiles = [None] * 12
                    psT_tiles = [None] * 6

                    def qk_unit(u):
                        pr, hh = u // 2, u % 2
                        hs = slice(hh * 64, hh * 64 + 64)
                        # attT [128, 8 kt, 512 q]
                        attT = attp.tile([128, NTT, TC], BF16, tag="attT",
                                         name="attT")
                        att_tiles[u] = attT
                        for k2 in range(4):          # kt pairs (2k2, 2k2+1)
                            c0 = k2 * 128            # q-col start (slot k2)
                            w = TC - c0
                            ps2 = sps.tile([128, 2, TC], F32, tag="sps", name="sps")
                            for i in range(2):
                                kt = 2 * k2 + i
                                nc.tensor.matmul(
                                    ps2[:, i, 0:w],
                                    kT_all[pr][hs, kt * 128:(kt + 1) * 128],
                                    qk_sb[pr][hs, c0:TC],
                                    start=True, stop=True)
                            nc.scalar.activation(
                                attT[:, 2 * k2:2 * k2 + 2, c0:TC],
                                ps2[:, :, 0:w], AF.Exp, scale=0.125)
                        for j in range(NSL):
                            # mask the (2j, 2j+1) key pair for query slot j
                            nc.gpsimd.tensor_mul(
                                attT[:, 2 * j:2 * j + 2, j * 128:(j + 1) * 128],
                                attT[:, 2 * j:2 * j + 2, j * 128:(j + 1) * 128],
                                md_sb)

                    def av_unit(u):
                        pr, hh = u // 2, u % 2
                        h = 2 * pr + hh
                        attT = att_tiles[u]
                        if hh == 0:
                            psT_tiles[pr] = tps.tile([128, TC], BF16, tag="psT",
                                                     name="psT")
                        psT = psT_tiles[pr]
                        for j in range(NSL):
                            po = ops.tile([128, 65], F32, tag="po", name="po")
                            for kt in range(2 * j + 2):
                                nc.tensor.matmul(
                                    po, attT[:, kt, j * 128:(j + 1) * 128],
                                    v_all[kt][:, h, :],
                                    start=(kt == 0), stop=(kt == 2 * j + 1))
                            r_sb = scratch.tile([128, 1], F32, tag="r_sb", name="r_sb")
                            ao = scratch.tile([128, 64], BF16, tag="ao", name="ao")
                            nc.vector.reciprocal(r_sb, po[:, 64:65])
                            nc.vector.tensor_scalar_mul(ao, po[:, 0:64], r_sb)
                            nc.tensor.transpose(
                                psT[hh * 64:hh * 64 + 64,
                                    j * 128:(j + 1) * 128],
                                ao, ident,
                                tile_position=(0, hh * 64))
                        if hh == 1:
                            nc.vector.tensor_copy(attoT[pr], psT)

                    qk_unit(0)
                    for u in range(12):
                        if u + 1 < 12:
                            qk_unit(u + 1)
                        av_unit(u)

                # ---------- proj + residual ----------
                with tc.tile_pool(name=f"pps{l}", bufs=4, space="PSUM") as pps:
                    for ot in range(NKT):
                        ps = pps.tile([128, TC], F32, tag="pps", name="pps")
                        for kt in range(NKT):
                            nc.tensor.matmul(
                                ps, pw_sb[kt][:, ot * 128:(ot + 1) * 128],
                                attoT[kt],
                                start=(kt == 0), stop=(kt == NKT - 1))
                        if ot % 2 == 0:
                            nc.vector.scalar_tensor_tensor(
                                xt[ot], ps, pb_sb[:, ot:ot + 1],
                                xt[ot], op0=ALU.add, op1=ALU.add)
                        else:
                            tmp = scratch.tile([128, TC], F32, tag="rtmp",
                                               name="rtmp")
                            nc.scalar.activation(tmp, ps, AF.Identity,
                                                 bias=pb_sb[:, ot:ot + 1])
                            nc.gpsimd.tensor_add(xt[ot], xt[ot], tmp)
                actx.close()

                # ---------- LN2 + MLP ----------
                h2in = [lnpool.tile([128, TC], BF16, tag=f"hbf{i}", name=f"hbf{i}")
                        for i in range(NKT)]

                with tc.tile_pool(name=f"stps{l}b", bufs=1, space="PSUM") as stats_ps, \
                     tc.tile_pool(name=f"mlpps{l}", bufs=3, space="PSUM") as mlpps, \
                     tc.tile_pool(name=f"mlpw{l}", bufs=1) as mlpw, \
                     tc.tile_pool(name=f"h2p{l}", bufs=1) as h2p:
                    fcw_sb = mlpw.tile([128, NKT, 4 * D], BF16, name="fcw_sb")
                    nc.sync.dma_start(fcw_sb,
                                      fcw_d[l].rearrange("(t p) f -> p t f", p=128))
                    fc2w_sb = mlpw.tile([128, 24, D], BF16, name="fc2w_sb")
                    nc.sync.dma_start(fc2w_sb,
                                      fc2w_d[l].rearrange("(t p) f -> p t f", p=128))
                    m0, r0 = _ln_stats(tc, nc, xt, small, scratch, stats_ps)
                    _ln_apply(tc, nc, xt, m0, r0, ln_sb[2], ln_sb[3], h2in, scratch)

                    h2c = [h2p.tile([128, TC], BF16, tag=f"h2c{f}", name=f"h2c{f}")
                           for f in range(24)]
                    for f in range(24):
                        ps = mlpps.tile([128, TC], F32, tag="fcps", name="fcps")
                        for kt in range(NKT):
                            nc.tensor.matmul(
                                ps, fcw_sb[:, kt, f * 128:(f + 1) * 128],
                                h2in[kt],
                                start=(kt == 0), stop=(kt == NKT - 1))
                        nc.scalar.activation(h2c[f], ps, AF.Gelu_apprx_tanh,
                                             bias=fcb_sb[:, f:f + 1])
                    for ot in range(NKT):
                        ps = mlpps.tile([128, TC], F32, tag="fc2ps", name="fc2ps")
                        for kt in range(24):
                            nc.tensor.matmul(ps, fc2w_sb[:, kt, ot * 128:(ot + 1) * 128],
                                             h2c[kt],
                                             start=(kt == 0), stop=(kt == 23))
                        if ot % 2 == 0:
                            nc.vector.scalar_tensor_tensor(
                                xt[ot], ps, fc2b_sb[:, ot:ot + 1],
                                xt[ot], op0=ALU.add, op1=ALU.add)
                        else:
                            tmp = scratch.tile([128, TC], F32, tag="rtmp",
                                               name="rtmp")
                            nc.scalar.activation(tmp, ps, AF.Identity,
                                                 bias=fc2b_sb[:, ot:ot + 1])
                            nc.gpsimd.tensor_add(xt[ot], xt[ot], tmp)

        # ---------- final LN + xf AllGather + lm_head ----------
        with ExitStack() as fctx:
            lnpool = fctx.enter_context(tc.tile_pool(name="lnfp", bufs=1))
            biasp = fctx.enter_context(tc.tile_pool(name="biasf", bufs=1))
            small = fctx.enter_context(tc.tile_pool(name="smallf", bufs=2))
            scratch = fctx.enter_context(tc.tile_pool(name="scrf", bufs=3))
            lmwp = fctx.enter_context(tc.tile_pool(name="lmw", bufs=4))

            lnfg_sb = biasp.tile([128, 6], F32)
            nc.sync.dma_start(lnfg_sb, lnf_d[0].rearrange("(t p) -> p t", p=128))
            lnfb_sb = biasp.tile([128, 6], F32)
            nc.sync.dma_start(lnfb_sb, lnf_d[1].rearrange("(t p) -> p t", p=128))

            lm_wt = {}
            def lm_fetch(vc):
                wt = lmwp.tile([128, NKT, 512], BF16, tag="lmw_t", name="lmw_t")
                nc.sync.dma_start(
                    wt, lmw_d[:, vc * 512:(vc + 1) * 512]
                    .rearrange("(t p) v -> p t v", p=128))
                lm_wt[vc] = wt
            lm_fetch(0)
            lm_fetch(1)

            xf_own = [lnpool.tile([128, TC], BF16, tag=f"xo{i}", name=f"xo{i}")
                      for i in range(NKT)]
            with tc.tile_pool(name="stpsf", bufs=1, space="PSUM") as stats_ps:
                m0, r0 = _ln_stats(tc, nc, xt, small, scratch, stats_ps)
                _ln_apply(tc, nc, xt, m0, r0, lnfg_sb, lnfb_sb, xf_own, scratch)

            xb_in = dram.tile([6, 128, TC], BF16, name="xb_in")
            xb_out = dram.tile([2, 6, 128, TC], BF16, name="xb_out")
            for i in range(6):
                nc.gpsimd.dma_start(xb_in[i], xf_own[i])
            nc.gpsimd.collective_compute(
                "AllGather", mybir.AluOpType.bypass,
                replica_groups=GROUPS,
                ins=[xb_in[:].opt()], outs=[xb_out[:].opt()])
            xf_all = [lnpool.tile([128, T], BF16, tag=f"xa{i}", name=f"xa{i}")
                      for i in range(NKT)]
            for f in range(NKT):
                for s in range(2):
                    dst = xf_all[f].rearrange(
                        "p (t two c) -> p t two c", two=2, c=128)[:, :, s, :]
                    nc.sync.dma_start(
                        dst, xb_out[s, f].rearrange("p (t c) -> p t c", c=128))

            with tc.tile_pool(name="lmps", bufs=4, space="PSUM") as lmps, \
                 tc.tile_pool(name="lmev", bufs=6) as lmev:
                for vc in range(NVC):
                    if vc + 2 < NVC:
                        lm_fetch(vc + 2)
                    wt = lm_wt.pop(vc)
                    for tt in range(NTT):
                        ps = lmps.tile([128, 512], F32, tag="lmps", name="lmps")
                        for kt in range(NKT):
                            nc.tensor.matmul(
                                ps, xf_all[kt][:, tt * 128:(tt + 1) * 128],
                                wt[:, kt, :],
                                start=(kt == 0), stop=(kt == NKT - 1))
                        ev = lmev.tile([128, 512], F16, tag="lmev", name="lmev")
                        if tt % 2 == 0:
                            nc.scalar.copy(ev, ps)
                        else:
                            nc.vector.tensor_copy(ev, ps)
                        nc.sync.dma_start(
                            out_d[tt * 128:(tt + 1) * 128,
                                  vc * 512:(vc + 1) * 512], ev)
    nc.finalize()
    return nc


_NC_CACHE = None


def _get_nc():
    global _NC_CACHE
    if _NC_CACHE is None:
        _NC_CACHE = build_bass()
    return _NC_CACHE


def make_in_maps(idx, layer_num, wte, wpe, ln1_g, ln1_b, attn_w, attn_b, proj_w,
                 proj_b, ln2_g, ln2_b, fc_w, fc_b, fc2_w, fc2_b, lnf_g, lnf_b, lm_w):
    bf = ml_dtypes.bfloat16
    idx = np.asarray(idx)
    f32 = np.float32
    wte = np.asarray(wte, f32)
    wpe = np.asarray(wpe, f32)
    x0 = wte[idx] + wpe[:T]                      # [B,T,D] fp32 host embedding

    attn_w = np.asarray(attn_w, f32)
    attn_b = np.asarray(attn_b, f32)
    proj_w = np.asarray(proj_w, f32)
    qkw = np.ascontiguousarray(attn_w[:, :, :2 * D]).astype(bf)
    vw = np.ascontiguousarray(attn_w[:, :, 2 * D:]).astype(bf)
    pw = proj_w.astype(bf)
    fcw = np.asarray(fc_w, f32).astype(bf)
    fc2w = np.asarray(fc2_w, f32).astype(bf)
    qkb = np.ascontiguousarray(attn_b[:, :2 * D])
    vb = np.ascontiguousarray(attn_b[:, 2 * D:])            # [L, D]
    pb_fold = np.einsum('ld,lde->le', vb, proj_w) + np.asarray(proj_b, f32)
    lnp = np.stack([np.asarray(ln1_g, f32), np.asarray(ln1_b, f32),
                    np.asarray(ln2_g, f32), np.asarray(ln2_b, f32)], axis=1)
    lnf = np.stack([np.asarray(lnf_g, f32), np.asarray(lnf_b, f32)], axis=0)

    lmw_pad = np.zeros((D, VPAD), f32)
    lmw_pad[:, :V] = np.asarray(lm_w, f32)
    lmw_bf = lmw_pad.astype(bf)

    tril = (np.arange(128)[:, None] <= np.arange(128)[None, :]).astype(np.float32)
    md = np.zeros((2, 128, 2, 128), np.float32)
    md[0, :, 0, :] = tril            # side 0: diag tile is its own slot tile
    md[0, :, 1, :] = 0.0             # side 0: extra odd tile fully masked
    md[1, :, 0, :] = 1.0             # side 1: even tile fully visible
    md[1, :, 1, :] = tril            # side 1: diag on the odd tile
    md = md.astype(bf)

    tok_idx = [np.concatenate([np.arange(128) + 128 * (2 * t + s)
                               for t in range(4)]) for s in range(2)]

    in_maps = []
    for core in range(8):
        b = core // 2
        s = core % 2
        vs = s * VSH
        in_maps.append(dict(
            xT=np.ascontiguousarray(x0[b][tok_idx[s]].T),
            qkw=qkw, vw=vw, pw=pw, fcw=fcw, fc2w=fc2w,
            qkb=qkb, pb=pb_fold,
            fcb=np.asarray(fc_b, f32), fc2b=np.asarray(fc2_b, f32),
            lnp=lnp, lnf=lnf, md=md[s],
            lmw=np.ascontiguousarray(lmw_bf[:, vs:vs + VSH]),
        ))
    return in_maps


def kernel(**inputs):
    global LAST_RESULT
    in_maps = make_in_maps(**inputs)
    nc = _get_nc()
    res = run_bass_kernel_spmd(nc, in_maps, core_ids=list(range(8)), trace=TRACE)
    LAST_RESULT = res

    logits = np.empty((B, T, V), np.float32)
    for b in range(B):
        lo = res.results[2 * b]["out"].astype(np.float32)
        hi = res.results[2 * b + 1]["out"].astype(np.float32)
        logits[b, :, :VSH] = lo
        logits[b, :, VSH:] = hi[:, :V - VSH]
    return logits


# revision 31
# speedup vs baseline: 1.3274x; 1.1492x over previous
"""GPT-2-ish forward (B=4, T=1024, D=768, H=12, L=2, V=50257) on 8 trn2 cores.

Sharding (v3): core pair (2b, 2b+1) sequence-splits the trunk for batch b.
Side s = core%2 owns the even (s=0) or odd (s=1) 128-token tiles of the
1024-token sequence: tiles {s, s+2, s+4, s+6} (interleaving balances the
causal attention load: key-tile needs sum to 16 vs 20). Each core runs
LN / qkv / proj / MLP on its own 512 tokens; K and V are pair-AllGathered
per layer (DRAM bounce) so attention sees all 1024 keys; the final-LN
output is pair-AllGathered before a vocab-split lm_head identical to v2
(each core: all 1024 tokens x 25600 vocab columns).

The SPMD program is identical on every core. Side-dependent causal
masking is data: slot j's last two key tiles (2j, 2j+1) are multiplied by
a per-core [128, 2, 128] mask md = (tril, zeros) on side 0 and
(ones, tril) on side 1. QK scores/exps only cover key tile kt from query
slot kt//2 onward; exp activations are issued per kt-pair on a
[128, 2, 512] PSUM tile to halve the activation-instruction count.

Engine layout as v2: PE ramps to 2.4 GHz only when streaming, so QK
matmuls of attention unit u+1 are issued before att@V of unit u; PSUM
evictions alternate DVE/scalar; GPSIMD (Pool, SBUF-only) takes masks,
LN subtract and residual adds. All matmuls bf16 with fp32 PSUM
accumulation; residual fp32; logits f16, upcast on host.
"""

import numpy as np
import ml_dtypes
from contextlib import ExitStack

import concourse.bass as bass
from concourse import bacc
import concourse.mybir as mybir
import concourse.tile as tile
from concourse.bass_utils import run_bass_kernel_spmd
from concourse.masks import make_identity

BF16 = mybir.dt.bfloat16
F32 = mybir.dt.float32
F16 = mybir.dt.float16
AF = mybir.ActivationFunctionType
ALU = mybir.AluOpType

V = 50257
VPAD = 51200          # 2 * 25600
VSH = VPAD // 2       # per-core vocab shard
D = 768
H = 12
HD = 64
L = 2
T = 1024
TC = 512              # tokens per core (trunk)
B = 4
EPS = 1e-5
NKT = D // 128        # 6 k-tiles over D
NTT = T // 128        # 8 global token-tiles
NSL = TC // 128       # 4 local token slots
NVC = VSH // 512      # 50 lm vocab chunks per core
GROUPS = [[0, 1], [2, 3], [4, 5], [6, 7]]

TRACE = False
LAST_RESULT = None

_SINGLES = {}


def _ln_stats(tc, nc, xt, small, scratch, stats_ps):
    """Stats over the core's 512 tokens: (mean, rstd) [128,512] fp32,
    broadcast across partitions."""
    onesq = _SINGLES["onesq"]
    eps128 = _SINGLES["eps128"]
    s1 = stats_ps.tile([128, TC], F32, tag="s1", name="s1")
    s2 = stats_ps.tile([128, TC], F32, tag="s2", name="s2")
    for kt in range(NKT):
        xbf = scratch.tile([128, TC], BF16, tag="xbf", name="xbf")
        sq = scratch.tile([128, TC], BF16, tag="sq", name="sq")
        nc.gpsimd.tensor_copy(xbf, xt[kt])
        nc.vector.tensor_mul(sq, xt[kt], xt[kt])
        nc.tensor.matmul(s1, onesq, xbf, start=(kt == 0), stop=(kt == NKT - 1))
        nc.tensor.matmul(s2, onesq, sq, start=(kt == 0), stop=(kt == NKT - 1))
    mean = small.tile([128, TC], F32, tag="mean", name="mean")
    rstd = small.tile([128, TC], F32, tag="rstd", name="rstd")
    var = scratch.tile([128, TC], F32, tag="var", name="var")
    nc.vector.tensor_scalar_mul(mean, s1, 1.0 / D)
    nc.vector.tensor_mul(var, mean, mean)
    nc.vector.scalar_tensor_tensor(var, s2, 1.0 / D, var,
                                   op0=ALU.mult, op1=ALU.subtract)
    nc.scalar.activation(var, var, AF.Sqrt, bias=eps128)
    nc.vector.reciprocal(rstd, var)
    return mean, rstd


def _ln_apply(tc, nc, xt, mean, rstd, g_sb, b_sb, out_tiles, scratch):
    for kt in range(NKT):
        t1 = scratch.tile([128, TC], F32, tag="lnt1", name="lnt1")
        eng = nc.gpsimd if kt % 2 == 0 else nc.vector
        eng.tensor_sub(t1, xt[kt], mean)
        nc.vector.scalar_tensor_tensor(t1, t1, g_sb[:, kt:kt + 1], rstd,
                                       op0=ALU.mult, op1=ALU.mult)
        nc.scalar.activation(out_tiles[kt], t1, AF.Identity,
                             bias=b_sb[:, kt:kt + 1])


def build_bass():
    nc = bacc.Bacc(None, target_bir_lowering=False)
    # ---- DRAM I/O (per-core shard views) ----
    xT_d = nc.dram_tensor("xT", [D, TC], F32, kind="ExternalInput")
    qkw_d = nc.dram_tensor("qkw", [L, D, 2 * D], BF16, kind="ExternalInput")
    vw_d = nc.dram_tensor("vw", [L, D, D], BF16, kind="ExternalInput")
    pw_d = nc.dram_tensor("pw", [L, D, D], BF16, kind="ExternalInput")
    fcw_d = nc.dram_tensor("fcw", [L, D, 4 * D], BF16, kind="ExternalInput")
    fc2w_d = nc.dram_tensor("fc2w", [L, 4 * D, D], BF16, kind="ExternalInput")
    qkb_d = nc.dram_tensor("qkb", [L, 2 * D], F32, kind="ExternalInput")
    pb_d = nc.dram_tensor("pb", [L, D], F32, kind="ExternalInput")
    fcb_d = nc.dram_tensor("fcb", [L, 4 * D], F32, kind="ExternalInput")
    fc2b_d = nc.dram_tensor("fc2b", [L, D], F32, kind="ExternalInput")
    ln_d = nc.dram_tensor("lnp", [L, 4, D], F32, kind="ExternalInput")
    lnf_d = nc.dram_tensor("lnf", [2, D], F32, kind="ExternalInput")
    md_d = nc.dram_tensor("md", [128, 2, 128], BF16, kind="ExternalInput")
    lmw_d = nc.dram_tensor("lmw", [D, VSH], BF16, kind="ExternalInput")
    out_d = nc.dram_tensor("out", [T, VSH], F16, kind="ExternalOutput")

    with tile.TileContext(nc) as tc, ExitStack() as octx:
        singles = octx.enter_context(tc.tile_pool(name="singles", bufs=1))
        resid = octx.enter_context(tc.tile_pool(name="resid", bufs=1))
        dram = octx.enter_context(tc.tile_pool(name="dram", bufs=1, space="DRAM"))

        onesq = singles.tile([128, 128], BF16)
        nc.vector.memset(onesq, 1.0)
        eps128 = singles.tile([128, 1], F32)
        nc.vector.memset(eps128, EPS)
        ident = singles.tile([128, 128], BF16)
        make_identity(nc, ident)
        _SINGLES["onesq"] = onesq
        _SINGLES["eps128"] = eps128

        md_sb = singles.tile([128, 2, 128], BF16)
        nc.sync.dma_start(md_sb, md_d[:, :, :])

        xt = [resid.tile([128, TC], F32, tag=f"xt{i}", name=f"xt{i}")
              for i in range(NKT)]
        for kt in range(NKT):
            nc.sync.dma_start(xt[kt], xT_d[kt * 128:(kt + 1) * 128, :])

        for l in range(L):
            with ExitStack() as lctx:
                lnpool = lctx.enter_context(tc.tile_pool(name=f"ln{l}", bufs=1))
                biasp = lctx.enter_context(tc.tile_pool(name=f"bias{l}", bufs=1))
                small = lctx.enter_context(tc.tile_pool(name=f"small{l}", bufs=2))
                scratch = lctx.enter_context(tc.tile_pool(name=f"scr{l}", bufs=3))
                actx = ExitStack()
                wpool = actx.enter_context(tc.tile_pool(name=f"w{l}", bufs=1))

                qkw_sb = wpool.tile([128, NKT, 2 * D], BF16, name="qkw_sb")
                nc.sync.dma_start(qkw_sb,
                                  qkw_d[l].rearrange("(t p) f -> p t f", p=128))
                vw_sb = [wpool.tile([128, D], BF16, tag=f"vw{i}", name=f"vw{i}")
                         for i in range(NKT)]
                for kt in range(NKT):
                    nc.sync.dma_start(vw_sb[kt], vw_d[l][kt * 128:(kt + 1) * 128, :])
                pw_sb = [wpool.tile([128, D], BF16, tag=f"pw{i}", name=f"pw{i}")
                        for i in range(NKT)]
                for kt in range(NKT):
                    nc.sync.dma_start(pw_sb[kt], pw_d[l][kt * 128:(kt + 1) * 128, :])

                qkb_sb = biasp.tile([128, 12], F32)
                nc.sync.dma_start(qkb_sb, qkb_d[l].rearrange("(t p) -> p t", p=128))
                pb_sb = biasp.tile([128, 6], F32)
                nc.sync.dma_start(pb_sb, pb_d[l].rearrange("(t p) -> p t", p=128))
                fcb_sb = biasp.tile([128, 24], F32)
                nc.sync.dma_start(fcb_sb, fcb_d[l].rearrange("(t p) -> p t", p=128))
                fc2b_sb = biasp.tile([128, 6], F32)
                nc.sync.dma_start(fc2b_sb, fc2b_d[l].rearrange("(t p) -> p t", p=128))
                ln_sb = []
                for i in range(4):
                    t = biasp.tile([128, 6], F32, tag=f"lnp{i}", name=f"lnp{i}")
                    nc.sync.dma_start(t, ln_d[l][i].rearrange("(t p) -> p t", p=128))
                    ln_sb.append(t)

                # ---------- LN1 ----------
                h_bf = [lnpool.tile([128, TC], BF16, tag=f"hbf{i}", name=f"hbf{i}")
                        for i in range(NKT)]
                qk_sb = [wpool.tile([128, TC], BF16, tag=f"qk{i}", name=f"qk{i}")
                         for i in range(12)]

                with tc.tile_pool(name=f"stps{l}a", bufs=1, space="PSUM") as stats_ps, \
                     tc.tile_pool(name=f"qkps{l}", bufs=3, space="PSUM") as qkps:
                    m0, r0 = _ln_stats(tc, nc, xt, small, scratch, stats_ps)
                    _ln_apply(tc, nc, xt, m0, r0, ln_sb[0], ln_sb[1], h_bf, scratch)

                    # K features first (f 6..11) so the K gather starts early
                    for f in list(range(6, 12)) + list(range(6)):
                        ps = qkps.tile([128, TC], F32, tag="qkps", name="qkps")
                        for kt in range(NKT):
                            nc.tensor.matmul(
                                ps, qkw_sb[:, kt, f * 128:(f + 1) * 128],
                                h_bf[kt],
                                start=(kt == 0), stop=(kt == NKT - 1))
                        nc.scalar.activation(qk_sb[f], ps, AF.Identity,
                                             bias=qkb_sb[:, f:f + 1])
                        if f == 8 or f == 11:
                            # ---- K AllGather (pair), split in two halves so
                            # the first head-pairs unblock early ----
                            half = 0 if f == 8 else 1
                            if half == 0:
                                kb_in = [dram.tile([3, 128, TC], BF16,
                                                   tag=f"kbi{l}{h}", name=f"kbi{l}{h}")
                                         for h in range(2)]
                                kb_out = [dram.tile([2, 3, 128, TC], BF16,
                                                    tag=f"kbo{l}{h}", name=f"kbo{l}{h}")
                                          for h in range(2)]
                            for i in range(3):
                                nc.gpsimd.dma_start(kb_in[half][i],
                                                    qk_sb[6 + 3 * half + i])
                            nc.gpsimd.collective_compute(
                                "AllGather", mybir.AluOpType.bypass,
                                replica_groups=GROUPS,
                                ins=[kb_in[half][:].opt()],
                                outs=[kb_out[half][:].opt()])

                    # ---------- V own tiles [128, 12, 65] (incl ones col) ----
                    v_own = [wpool.tile([128, 12, 65], BF16, tag=f"vown{i}",
                                        name=f"vown{i}") for i in range(NSL)]
                    for tt in range(NSL):
                        nc.vector.memset(v_own[tt][:, :, 64:65], 1.0)
                        for vc in range(2):
                            vs = slice(vc * 384, (vc + 1) * 384)
                            ps = qkps.tile([128, 384], F32, tag="vps", name="vps")
                            for kt in range(NKT):
                                nc.tensor.matmul(
                                    ps, h_bf[kt][:, tt * 128:(tt + 1) * 128],
                                    vw_sb[kt][:, vs],
                                    start=(kt == 0), stop=(kt == NKT - 1))
                            nc.vector.tensor_copy(
                                v_own[tt][:, vc * 6:(vc + 1) * 6, 0:64],
                                ps.rearrange("p (h d) -> p h d", d=64))

                    # ---- V AllGather (pair), split in two halves ----
                    vb_in = [dram.tile([2, 128, 12 * 65], BF16, tag=f"vbi{l}{h}",
                                       name=f"vbi{l}{h}") for h in range(2)]
                    vb_out = [dram.tile([2, 2, 128, 12 * 65], BF16, tag=f"vbo{l}{h}",
                                        name=f"vbo{l}{h}") for h in range(2)]
                    for h in range(2):
                        for i in range(2):
                            nc.gpsimd.dma_start(
                                vb_in[h][i],
                                v_own[2 * h + i].rearrange("p h d -> p (h d)"))
                        nc.gpsimd.collective_compute(
                            "AllGather", mybir.AluOpType.bypass,
                            replica_groups=GROUPS,
                            ins=[vb_in[h][:].opt()], outs=[vb_out[h][:].opt()])

                    # ---- gathered K back to SBUF: kT_all[f] [128, 1024] ----
                    kT_all = [wpool.tile([128, T], BF16, tag=f"kta{i}",
                                         name=f"kta{i}") for i in range(6)]
                    for f in range(6):
                        for s in range(2):
                            dst = kT_all[f].rearrange(
                                "p (t two c) -> p t two c", two=2, c=128)[:, :, s, :]
                            nc.sync.dma_start(
                                dst, kb_out[f // 3][s, f % 3].rearrange(
                                    "p (t c) -> p t c", c=128))
                    # ---- gathered V back: v_all[g] [128, 12, 65] ----
                    # local tile t of side s = global tile 2t+s; halves h by t//2
                    v_all = [wpool.tile([128, 12, 65], BF16, tag=f"vall{i}",
                                        name=f"vall{i}") for i in range(NTT)]
                    for g in range(NTT):
                        s, t = g % 2, g // 2
                        nc.sync.dma_start(
                            v_all[g],
                            vb_out[t // 2][s, t % 2].rearrange("p (h d) -> p h d", d=65))

                # ---------- attention: 12 (pair, half) units, pipelined ----
                attoT = [lnpool.tile([128, TC], BF16, tag=f"attoT{i}", name=f"attoT{i}")
                         for i in range(NKT)]
                # prefetch fc weights during attention (layer-long pool)
                fcw_sb = lnpool.tile([128, NKT, 4 * D], BF16, name="fcw_sb")
                nc.sync.dma_start(fcw_sb,
                                  fcw_d[l].rearrange("(t p) f -> p t f", p=128))
                with tc.tile_pool(name=f"sps{l}", bufs=2, space="PSUM") as sps, \
                     tc.tile_pool(name=f"ops{l}", bufs=2, space="PSUM") as ops, \
                     tc.tile_pool(name=f"tps{l}", bufs=2, space="PSUM") as tps, \
                     tc.tile_pool(name=f"attp{l}", bufs=3) as attp:

                    att_tiles = [None] * 12
                    psT_tiles = [None] * 6

                    def qk_unit(u):
                        pr, hh = u // 2, u % 2
                        hs = slice(hh * 64, hh * 64 + 64)
                        # attT [128, 8 kt, 512 q]
                        attT = attp.tile([128, NTT, TC], BF16, tag="attT",
                                         name="attT")
                        att_tiles[u] = attT
                        for k2 in range(4):          # kt pairs (2k2, 2k2+1)
                            c0 = k2 * 128            # q-col start (slot k2)
                            w = TC - c0
                            ps2 = sps.tile([128, 2, TC], F32, tag="sps", name="sps")
                            for i in range(2):
                                kt = 2 * k2 + i
                                nc.tensor.matmul(
                                    ps2[:, i, 0:w],
                                    kT_all[pr][hs, kt * 128:(kt + 1) * 128],
                                    qk_sb[pr][hs, c0:TC],
                                    start=True, stop=True)
                            nc.scalar.activation(
                                attT[:, 2 * k2:2 * k2 + 2, c0:TC],
                                ps2[:, :, 0:w], AF.Exp, scale=0.125)
                        # gpsimd is busy with the K/V collectives early on
                        meng = nc.vector if u < 4 else nc.gpsimd
                        for j in range(NSL):
                            # mask the (2j, 2j+1) key pair for query slot j
                            meng.tensor_mul(
                                attT[:, 2 * j:2 * j + 2, j * 128:(j + 1) * 128],
                                attT[:, 2 * j:2 * j + 2, j * 128:(j + 1) * 128],
                                md_sb)

                    def av_unit(u):
                        pr, hh = u // 2, u % 2
                        h = 2 * pr + hh
                        attT = att_tiles[u]
                        if hh == 0:
                            psT_tiles[pr] = tps.tile([128, TC], BF16, tag="psT",
                                                     name="psT")
                        psT = psT_tiles[pr]
                        for j in range(NSL):
                            po = ops.tile([128, 65], F32, tag="po", name="po")
                            for kt in range(2 * j + 2):
                                nc.tensor.matmul(
                                    po, attT[:, kt, j * 128:(j + 1) * 128],
                                    v_all[kt][:, h, :],
                                    start=(kt == 0), stop=(kt == 2 * j + 1))
                            r_sb = scratch.tile([128, 1], F32, tag="r_sb", name="r_sb")
                            ao = scratch.tile([128, 64], BF16, tag="ao", name="ao")
                            nc.vector.reciprocal(r_sb, po[:, 64:65])
                            nc.vector.tensor_scalar_mul(ao, po[:, 0:64], r_sb)
                            nc.tensor.transpose(
                                psT[hh * 64:hh * 64 + 64,
                                    j * 128:(j + 1) * 128],
                                ao, ident,
                                tile_position=(0, hh * 64))
                        if hh == 1:
                            nc.vector.tensor_copy(attoT[pr], psT)

                    qk_unit(0)
                    qk_unit(1)
                    for u in range(12):
                        if u + 2 < 12:
                            qk_unit(u + 2)
                        av_unit(u)

                # ---------- proj + residual ----------
                with tc.tile_pool(name=f"pps{l}", bufs=4, space="PSUM") as pps:
                    for ot in range(NKT):
                        ps = pps.tile([128, TC], F32, tag="pps", name="pps")
                        for kt in range(NKT):
                            nc.tensor.matmul(
                                ps, pw_sb[kt][:, ot * 128:(ot + 1) * 128],
                                attoT[kt],
                                start=(kt == 0), stop=(kt == NKT - 1))
                        if ot % 2 == 0:
                            nc.vector.scalar_tensor_tensor(
                                xt[ot], ps, pb_sb[:, ot:ot + 1],
                                xt[ot], op0=ALU.add, op1=ALU.add)
                        else:
                            tmp = scratch.tile([128, TC], F32, tag="rtmp",
                                               name="rtmp")
                            nc.scalar.activation(tmp, ps, AF.Identity,
                                                 bias=pb_sb[:, ot:ot + 1])
                            nc.gpsimd.tensor_add(xt[ot], xt[ot], tmp)
                actx.close()

                # ---------- LN2 + MLP ----------
                h2in = [lnpool.tile([128, TC], BF16, tag=f"hbf{i}", name=f"hbf{i}")
                        for i in range(NKT)]

                with tc.tile_pool(name=f"stps{l}b", bufs=1, space="PSUM") as stats_ps, \
                     tc.tile_pool(name=f"mlpps{l}", bufs=3, space="PSUM") as mlpps, \
                     tc.tile_pool(name=f"mlpw{l}", bufs=1) as mlpw, \
                     tc.tile_pool(name=f"h2p{l}", bufs=1) as h2p:
                    fc2w_sb = mlpw.tile([128, 24, D], BF16, name="fc2w_sb")
                    nc.sync.dma_start(fc2w_sb,
                                      fc2w_d[l].rearrange("(t p) f -> p t f", p=128))
                    m0, r0 = _ln_stats(tc, nc, xt, small, scratch, stats_ps)
                    _ln_apply(tc, nc, xt, m0, r0, ln_sb[2], ln_sb[3], h2in, scratch)

                    h2c = [h2p.tile([128, TC], BF16, tag=f"h2c{f}", name=f"h2c{f}")
                           for f in range(24)]
                    for f in range(24):
                        ps = mlpps.tile([128, TC], F32, tag="fcps", name="fcps")
                        for kt in range(NKT):
                            nc.tensor.matmul(
                                ps, fcw_sb[:, kt, f * 128:(f + 1) * 128],
                                h2in[kt],
                                start=(kt == 0), stop=(kt == NKT - 1))
                        nc.scalar.activation(h2c[f], ps, AF.Gelu_apprx_tanh,
                                             bias=fcb_sb[:, f:f + 1])
                    for ot in range(NKT):
                        ps = mlpps.tile([128, TC], F32, tag="fc2ps", name="fc2ps")
                        for kt in range(24):
                            nc.tensor.matmul(ps, fc2w_sb[:, kt, ot * 128:(ot + 1) * 128],
                                             h2c[kt],
                                             start=(kt == 0), stop=(kt == 23))
                        if ot % 2 == 0:
                            nc.vector.scalar_tensor_tensor(
                                xt[ot], ps, fc2b_sb[:, ot:ot + 1],
                                xt[ot], op0=ALU.add, op1=ALU.add)
                        else:
                            tmp = scratch.tile([128, TC], F32, tag="rtmp",
                                               name="rtmp")
                            nc.scalar.activation(tmp, ps, AF.Identity,
                                                 bias=fc2b_sb[:, ot:ot + 1])
                            nc.gpsimd.tensor_add(xt[ot], xt[ot], tmp)

        # ---------- final LN + xf AllGather + lm_head ----------
        with ExitStack() as fctx:
            lnpool = fctx.enter_context(tc.tile_pool(name="lnfp", bufs=1))
            biasp = fctx.enter_context(tc.tile_pool(name="biasf", bufs=1))
            small = fctx.enter_context(tc.tile_pool(name="smallf", bufs=2))
            scratch = fctx.enter_context(tc.tile_pool(name="scrf", bufs=3))
            lmwp = fctx.enter_context(tc.tile_pool(name="lmw", bufs=4))

            lnfg_sb = biasp.tile([128, 6], F32)
            nc.sync.dma_start(lnfg_sb, lnf_d[0].rearrange("(t p) -> p t", p=128))
            lnfb_sb = biasp.tile([128, 6], F32)
            nc.sync.dma_start(lnfb_sb, lnf_d[1].rearrange("(t p) -> p t", p=128))

            lm_wt = {}
            def lm_fetch(vc):
                wt = lmwp.tile([128, NKT, 512], BF16, tag="lmw_t", name="lmw_t")
                nc.sync.dma_start(
                    wt, lmw_d[:, vc * 512:(vc + 1) * 512]
                    .rearrange("(t p) v -> p t v", p=128))
                lm_wt[vc] = wt
            lm_fetch(0)
            lm_fetch(1)

            xf_own = [lnpool.tile([128, TC], BF16, tag=f"xo{i}", name=f"xo{i}")
                      for i in range(NKT)]
            with tc.tile_pool(name="stpsf", bufs=1, space="PSUM") as stats_ps:
                m0, r0 = _ln_stats(tc, nc, xt, small, scratch, stats_ps)
                _ln_apply(tc, nc, xt, m0, r0, lnfg_sb, lnfb_sb, xf_own, scratch)

            xb_in = [dram.tile([3, 128, TC], BF16, tag=f"xbi{h}", name=f"xbi{h}")
                     for h in range(2)]
            xb_out = [dram.tile([2, 3, 128, TC], BF16, tag=f"xbo{h}", name=f"xbo{h}")
                      for h in range(2)]
            xf_all = [lnpool.tile([128, T], BF16, tag=f"xa{i}", name=f"xa{i}")
                      for i in range(NKT)]
            for h in range(2):
                for i in range(3):
                    nc.gpsimd.dma_start(xb_in[h][i], xf_own[3 * h + i])
                nc.gpsimd.collective_compute(
                    "AllGather", mybir.AluOpType.bypass,
                    replica_groups=GROUPS,
                    ins=[xb_in[h][:].opt()], outs=[xb_out[h][:].opt()])
                for i in range(3):
                    f = 3 * h + i
                    for s in range(2):
                        dst = xf_all[f].rearrange(
                            "p (t two c) -> p t two c", two=2, c=128)[:, :, s, :]
                        nc.sync.dma_start(
                            dst, xb_out[h][s, i].rearrange("p (t c) -> p t c", c=128))

            with tc.tile_pool(name="lmps", bufs=4, space="PSUM") as lmps, \
                 tc.tile_pool(name="lmev", bufs=6) as lmev:
                for vc in range(NVC):
                    if vc + 2 < NVC:
                        lm_fetch(vc + 2)
                    wt = lm_wt.pop(vc)
                    for tt in range(NTT):
                        ps = lmps.tile([128, 512], F32, tag="lmps", name="lmps")
                        for kt in range(NKT):
                            nc.tensor.matmul(
                                ps, xf_all[kt][:, tt * 128:(tt + 1) * 128],
                                wt[:, kt, :],
                                start=(kt == 0), stop=(kt == NKT - 1))
                        ev = lmev.tile([128, 512], F16, tag="lmev", name="lmev")
                        if tt % 2 == 0:
                            nc.scalar.copy(ev, ps)
                        else:
                            nc.vector.tensor_copy(ev, ps)
                        nc.sync.dma_start(
                            out_d[tt * 128:(tt + 1) * 128,
                                  vc * 512:(vc + 1) * 512], ev)
    nc.finalize()
    return nc


_NC_CACHE = None


def _get_nc():
    global _NC_CACHE
    if _NC_CACHE is None:
        _NC_CACHE = build_bass()
    return _NC_CACHE


def make_in_maps(idx, layer_num, wte, wpe, ln1_g, ln1_b, attn_w, attn_b, proj_w,
                 proj_b, ln2_g, ln2_b, fc_w, fc_b, fc2_w, fc2_b, lnf_g, lnf_b, lm_w):
    bf = ml_dtypes.bfloat16
    idx = np.asarray(idx)
    f32 = np.float32
    wte = np.asarray(wte, f32)
    wpe = np.asarray(wpe, f32)
    x0 = wte[idx] + wpe[:T]                      # [B,T,D] fp32 host embedding

    attn_w = np.asarray(attn_w, f32)
    attn_b = np.asarray(attn_b, f32)
    proj_w = np.asarray(proj_w, f32)
    qkw = np.ascontiguousarray(attn_w[:, :, :2 * D]).astype(bf)
    vw = np.ascontiguousarray(attn_w[:, :, 2 * D:]).astype(bf)
    pw = proj_w.astype(bf)
    fcw = np.asarray(fc_w, f32).astype(bf)
    fc2w = np.asarray(fc2_w, f32).astype(bf)
    qkb = np.ascontiguousarray(attn_b[:, :2 * D])
    vb = np.ascontiguousarray(attn_b[:, 2 * D:])            # [L, D]
    pb_fold = np.einsum('ld,lde->le', vb, proj_w) + np.asarray(proj_b, f32)
    lnp = np.stack([np.asarray(ln1_g, f32), np.asarray(ln1_b, f32),
                    np.asarray(ln2_g, f32), np.asarray(ln2_b, f32)], axis=1)
    lnf = np.stack([np.asarray(lnf_g, f32), np.asarray(lnf_b, f32)], axis=0)

    lmw_pad = np.zeros((D, VPAD), f32)
    lmw_pad[:, :V] = np.asarray(lm_w, f32)
    lmw_bf = lmw_pad.astype(bf)

    tril = (np.arange(128)[:, None] <= np.arange(128)[None, :]).astype(np.float32)
    md = np.zeros((2, 128, 2, 128), np.float32)
    md[0, :, 0, :] = tril            # side 0: diag tile is its own slot tile
    md[0, :, 1, :] = 0.0             # side 0: extra odd tile fully masked
    md[1, :, 0, :] = 1.0             # side 1: even tile fully visible
    md[1, :, 1, :] = tril            # side 1: diag on the odd tile
    md = md.astype(bf)

    tok_idx = [np.concatenate([np.arange(128) + 128 * (2 * t + s)
                               for t in range(4)]) for s in range(2)]

    in_maps = []
    for core in range(8):
        b = core // 2
        s = core % 2
        vs = s * VSH
        in_maps.append(dict(
            xT=np.ascontiguousarray(x0[b][tok_idx[s]].T),
            qkw=qkw, vw=vw, pw=pw, fcw=fcw, fc2w=fc2w,
            qkb=qkb, pb=pb_fold,
            fcb=np.asarray(fc_b, f32), fc2b=np.asarray(fc2_b, f32),
            lnp=lnp, lnf=lnf, md=md[s],
            lmw=np.ascontiguousarray(lmw_bf[:, vs:vs + VSH]),
        ))
    return in_maps


def kernel(**inputs):
    global LAST_RESULT
    in_maps = make_in_maps(**inputs)
    nc = _get_nc()
    res = run_bass_kernel_spmd(nc, in_maps, core_ids=list(range(8)), trace=TRACE)
    LAST_RESULT = res

    logits = np.empty((B, T, V), np.float32)
    for b in range(B):
        lo = res.results[2 * b]["out"].astype(np.float32)
        hi = res.results[2 * b + 1]["out"].astype(np.float32)
        logits[b, :, :VSH] = lo
        logits[b, :, VSH:] = hi[:, :V - VSH]
    return logits


# revision 40
# speedup vs baseline: 1.3384x; 1.0083x over previous
"""GPT-2-ish forward (B=4, T=1024, D=768, H=12, L=2, V=50257) on 8 trn2 cores.

Sharding (v3): core pair (2b, 2b+1) sequence-splits the trunk for batch b.
Side s = core%2 owns the even (s=0) or odd (s=1) 128-token tiles of the
1024-token sequence: tiles {s, s+2, s+4, s+6} (interleaving balances the
causal attention load: key-tile needs sum to 16 vs 20). Each core runs
LN / qkv / proj / MLP on its own 512 tokens; K and V are pair-AllGathered
per layer (DRAM bounce) so attention sees all 1024 keys; the final-LN
output is pair-AllGathered before a vocab-split lm_head identical to v2
(each core: all 1024 tokens x 25600 vocab columns).

The SPMD program is identical on every core. Side-dependent causal
masking is data: slot j's last two key tiles (2j, 2j+1) are multiplied by
a per-core [128, 2, 128] mask md = (tril, zeros) on side 0 and
(ones, tril) on side 1. QK scores/exps only cover key tile kt from query
slot kt//2 onward; exp activations are issued per kt-pair on a
[128, 2, 512] PSUM tile to halve the activation-instruction count.

Engine layout as v2: PE ramps to 2.4 GHz only when streaming, so QK
matmuls of attention unit u+1 are issued before att@V of unit u; PSUM
evictions alternate DVE/scalar; GPSIMD (Pool, SBUF-only) takes masks,
LN subtract and residual adds. All matmuls bf16 with fp32 PSUM
accumulation; residual fp32; logits f16, upcast on host.
"""

import numpy as np
import ml_dtypes
from contextlib import ExitStack

import concourse.bass as bass
from concourse import bacc
import concourse.mybir as mybir
import concourse.tile as tile
from concourse.bass_utils import run_bass_kernel_spmd
from concourse.masks import make_identity

BF16 = mybir.dt.bfloat16
F32 = mybir.dt.float32
F16 = mybir.dt.float16
AF = mybir.ActivationFunctionType
ALU = mybir.AluOpType

V = 50257
VPAD = 51200          # 2 * 25600
VSH = VPAD // 2       # per-core vocab shard
D = 768
H = 12
HD = 64
L = 2
T = 1024
TC = 512              # tokens per core (trunk)
B = 4
EPS = 1e-5
NKT = D // 128        # 6 k-tiles over D
NTT = T // 128        # 8 global token-tiles
NSL = TC // 128       # 4 local token slots
NVC = VSH // 512      # 50 lm vocab chunks per core
GROUPS = [[0, 1], [2, 3], [4, 5], [6, 7]]

TRACE = False
LAST_RESULT = None

_SINGLES = {}


def _ln_stats_open(stats_ps):
    s1 = stats_ps.tile([128, TC], F32, tag="s1", name="s1")
    s2 = stats_ps.tile([128, TC], F32, tag="s2", name="s2")
    return s1, s2


def _ln_stats_accum(nc, s12, kt, x_tile, scratch):
    """Accumulate sum / sum-of-squares of one feature tile into PSUM.
    Emitted right after the residual update of that tile so the LN of the
    next sublayer has no serial stats phase."""
    s1, s2 = s12
    onesq = _SINGLES["onesq"]
    xbf = scratch.tile([128, TC], BF16, tag="xbf", name="xbf")
    sq = scratch.tile([128, TC], BF16, tag="sq", name="sq")
    nc.gpsimd.tensor_copy(xbf, x_tile)
    nc.vector.tensor_mul(sq, x_tile, x_tile)
    nc.tensor.matmul(s1, onesq, xbf, start=(kt == 0), stop=(kt == NKT - 1))
    nc.tensor.matmul(s2, onesq, sq, start=(kt == 0), stop=(kt == NKT - 1))


def _ln_stats_finish(nc, s12, small, scratch):
    s1, s2 = s12
    eps128 = _SINGLES["eps128"]
    mean = small.tile([128, TC], F32, tag="mean", name="mean")
    rstd = small.tile([128, TC], F32, tag="rstd", name="rstd")
    var = scratch.tile([128, TC], F32, tag="var", name="var")
    nc.vector.tensor_scalar_mul(mean, s1, 1.0 / D)
    nc.vector.tensor_mul(var, mean, mean)
    nc.vector.scalar_tensor_tensor(var, s2, 1.0 / D, var,
                                   op0=ALU.mult, op1=ALU.subtract)
    nc.scalar.activation(var, var, AF.Sqrt, bias=eps128)
    nc.vector.reciprocal(rstd, var)
    return mean, rstd


def _ln_stats(tc, nc, xt, small, scratch, stats_ps):
    """Standalone stats (used when there is no preceding loop to fuse into)."""
    s12 = _ln_stats_open(stats_ps)
    for kt in range(NKT):
        _ln_stats_accum(nc, s12, kt, xt[kt], scratch)
    return _ln_stats_finish(nc, s12, small, scratch)


def _ln_apply(tc, nc, xt, mean, rstd, g_sb, b_sb, out_tiles, scratch):
    for kt in range(NKT):
        t1 = scratch.tile([128, TC], F32, tag="lnt1", name="lnt1")
        eng = nc.gpsimd if kt % 2 == 0 else nc.vector
        eng.tensor_sub(t1, xt[kt], mean)
        nc.vector.scalar_tensor_tensor(t1, t1, g_sb[:, kt:kt + 1], rstd,
                                       op0=ALU.mult, op1=ALU.mult)
        nc.scalar.activation(out_tiles[kt], t1, AF.Identity,
                             bias=b_sb[:, kt:kt + 1])


def build_bass():
    nc = bacc.Bacc(None, target_bir_lowering=False)
    # ---- DRAM I/O (per-core shard views) ----
    xT_d = nc.dram_tensor("xT", [D, TC], F32, kind="ExternalInput")
    qkw_d = nc.dram_tensor("qkw", [L, D, 2 * D], BF16, kind="ExternalInput")
    vw_d = nc.dram_tensor("vw", [L, D, D], BF16, kind="ExternalInput")
    pw_d = nc.dram_tensor("pw", [L, D, D], BF16, kind="ExternalInput")
    fcw_d = nc.dram_tensor("fcw", [L, D, 4 * D], BF16, kind="ExternalInput")
    fc2w_d = nc.dram_tensor("fc2w", [L, 4 * D, D], BF16, kind="ExternalInput")
    qkb_d = nc.dram_tensor("qkb", [L, 2 * D], F32, kind="ExternalInput")
    pb_d = nc.dram_tensor("pb", [L, D], F32, kind="ExternalInput")
    fcb_d = nc.dram_tensor("fcb", [L, 4 * D], F32, kind="ExternalInput")
    fc2b_d = nc.dram_tensor("fc2b", [L, D], F32, kind="ExternalInput")
    ln_d = nc.dram_tensor("lnp", [L, 4, D], F32, kind="ExternalInput")
    lnf_d = nc.dram_tensor("lnf", [2, D], F32, kind="ExternalInput")
    md_d = nc.dram_tensor("md", [128, 2, 128], BF16, kind="ExternalInput")
    lmw_d = nc.dram_tensor("lmw", [D, VSH], BF16, kind="ExternalInput")
    out_d = nc.dram_tensor("out", [T, VSH], F16, kind="ExternalOutput")

    with tile.TileContext(nc) as tc, ExitStack() as octx:
        singles = octx.enter_context(tc.tile_pool(name="singles", bufs=1))
        resid = octx.enter_context(tc.tile_pool(name="resid", bufs=1))
        dram = octx.enter_context(tc.tile_pool(name="dram", bufs=1, space="DRAM"))

        onesq = singles.tile([128, 128], BF16)
        nc.vector.memset(onesq, 1.0)
        eps128 = singles.tile([128, 1], F32)
        nc.vector.memset(eps128, EPS)
        ident = singles.tile([128, 128], BF16)
        make_identity(nc, ident)
        _SINGLES["onesq"] = onesq
        _SINGLES["eps128"] = eps128

        md_sb = singles.tile([128, 2, 128], BF16)
        nc.sync.dma_start(md_sb, md_d[:, :, :])

        small = octx.enter_context(tc.tile_pool(name="small", bufs=2))

        # embedding load with LN1 stats fused in (no serial stats phase)
        xt = [resid.tile([128, TC], F32, tag=f"xt{i}", name=f"xt{i}")
              for i in range(NKT)]
        with tc.tile_pool(name="scr_init", bufs=3) as scr0, \
             tc.tile_pool(name="stps_init", bufs=1, space="PSUM") as stps0:
            s12 = _ln_stats_open(stps0)
            for kt in range(NKT):
                nc.sync.dma_start(xt[kt], xT_d[kt * 128:(kt + 1) * 128, :])
                _ln_stats_accum(nc, s12, kt, xt[kt], scr0)
            pending_stats = _ln_stats_finish(nc, s12, small, scr0)

        for l in range(L):
            with ExitStack() as lctx:
                lnpool = lctx.enter_context(tc.tile_pool(name=f"ln{l}", bufs=1))
                biasp = lctx.enter_context(tc.tile_pool(name=f"bias{l}", bufs=1))
                scratch = lctx.enter_context(tc.tile_pool(name=f"scr{l}", bufs=3))
                actx = ExitStack()
                wpool = actx.enter_context(tc.tile_pool(name=f"w{l}", bufs=1))

                qkw_sb = wpool.tile([128, NKT, 2 * D], BF16, name="qkw_sb")
                nc.sync.dma_start(qkw_sb,
                                  qkw_d[l].rearrange("(t p) f -> p t f", p=128))
                vw_sb = [wpool.tile([128, D], BF16, tag=f"vw{i}", name=f"vw{i}")
                         for i in range(NKT)]
                for kt in range(NKT):
                    nc.sync.dma_start(vw_sb[kt], vw_d[l][kt * 128:(kt + 1) * 128, :])
                pw_sb = [wpool.tile([128, D], BF16, tag=f"pw{i}", name=f"pw{i}")
                         for i in range(NKT)]
                for kt in range(NKT):
                    nc.sync.dma_start(pw_sb[kt], pw_d[l][kt * 128:(kt + 1) * 128, :])

                qkb_sb = biasp.tile([128, 12], F32)
                nc.sync.dma_start(qkb_sb, qkb_d[l].rearrange("(t p) -> p t", p=128))
                pb_sb = biasp.tile([128, 6], F32)
                nc.sync.dma_start(pb_sb, pb_d[l].rearrange("(t p) -> p t", p=128))
                fcb_sb = biasp.tile([128, 24], F32)
                nc.sync.dma_start(fcb_sb, fcb_d[l].rearrange("(t p) -> p t", p=128))
                fc2b_sb = biasp.tile([128, 6], F32)
                nc.sync.dma_start(fc2b_sb, fc2b_d[l].rearrange("(t p) -> p t", p=128))
                ln_sb = []
                for i in range(4):
                    t = biasp.tile([128, 6], F32, tag=f"lnp{i}", name=f"lnp{i}")
                    nc.sync.dma_start(t, ln_d[l][i].rearrange("(t p) -> p t", p=128))
                    ln_sb.append(t)

                # ---------- LN1 ----------
                h_bf = [lnpool.tile([128, TC], BF16, tag=f"hbf{i}", name=f"hbf{i}")
                        for i in range(NKT)]
                qk_sb = [wpool.tile([128, TC], BF16, tag=f"qk{i}", name=f"qk{i}")
                         for i in range(12)]

                kb_in = [dram.tile([3, 128, TC], BF16,
                                   tag=f"kbi{l}{h}", name=f"kbi{l}{h}")
                         for h in range(2)]
                kb_out = [dram.tile([2, 3, 128, TC], BF16,
                                    tag=f"kbo{l}{h}", name=f"kbo{l}{h}")
                          for h in range(2)]

                def k_gather(half):
                    for i in range(3):
                        nc.gpsimd.dma_start(kb_in[half][i],
                                            qk_sb[6 + 3 * half + i])
                    nc.gpsimd.collective_compute(
                        "AllGather", mybir.AluOpType.bypass,
                        replica_groups=GROUPS,
                        ins=[kb_in[half][:].opt()],
                        outs=[kb_out[half][:].opt()])

                with tc.tile_pool(name=f"stps{l}a", bufs=1, space="PSUM") as stats_ps, \
                     tc.tile_pool(name=f"qkps{l}", bufs=3, space="PSUM") as qkps:
                    if pending_stats is None:
                        m0, r0 = _ln_stats(tc, nc, xt, small, scratch, stats_ps)
                    else:
                        m0, r0 = pending_stats
                    _ln_apply(tc, nc, xt, m0, r0, ln_sb[0], ln_sb[1], h_bf, scratch)

                    # K features first (f 6..11) so the K gather starts early
                    for f in list(range(6, 12)) + list(range(6)):
                        ps = qkps.tile([128, TC], F32, tag="qkps", name="qkps")
                        for kt in range(NKT):
                            nc.tensor.matmul(
                                ps, qkw_sb[:, kt, f * 128:(f + 1) * 128],
                                h_bf[kt],
                                start=(kt == 0), stop=(kt == NKT - 1))
                        nc.scalar.activation(qk_sb[f], ps, AF.Identity,
                                             bias=qkb_sb[:, f:f + 1])
                        if f == 8:
                            k_gather(0)   # covers head-pairs 0-2

                    # ---------- V own tiles [128, 12, 65] (incl ones col) ----
                    # V gather halves are emitted mid-loop: half h covers
                    # global key tiles 4h..4h+3, needed by att@V slots 2h..
                    vb_in = [dram.tile([2, 128, 12 * 65], BF16, tag=f"vbi{l}{h}",
                                       name=f"vbi{l}{h}") for h in range(2)]
                    vb_out = [dram.tile([2, 2, 128, 12 * 65], BF16, tag=f"vbo{l}{h}",
                                        name=f"vbo{l}{h}") for h in range(2)]
                    v_own = [wpool.tile([128, 12, 65], BF16, tag=f"vown{i}",
                                        name=f"vown{i}") for i in range(NSL)]
                    for tt in range(NSL):
                        nc.vector.memset(v_own[tt][:, :, 64:65], 1.0)
                        for vc in range(2):
                            vs = slice(vc * 384, (vc + 1) * 384)
                            ps = qkps.tile([128, 384], F32, tag="vps", name="vps")
                            for kt in range(NKT):
                                nc.tensor.matmul(
                                    ps, h_bf[kt][:, tt * 128:(tt + 1) * 128],
                                    vw_sb[kt][:, vs],
                                    start=(kt == 0), stop=(kt == NKT - 1))
                            nc.vector.tensor_copy(
                                v_own[tt][:, vc * 6:(vc + 1) * 6, 0:64],
                                ps.rearrange("p (h d) -> p h d", d=64))
                        if tt % 2 == 1:
                            h = tt // 2
                            for i in range(2):
                                nc.gpsimd.dma_start(
                                    vb_in[h][i],
                                    v_own[2 * h + i].rearrange("p h d -> p (h d)"))
                            nc.gpsimd.collective_compute(
                                "AllGather", mybir.AluOpType.bypass,
                                replica_groups=GROUPS,
                                ins=[vb_in[h][:].opt()], outs=[vb_out[h][:].opt()])
                    k_gather(1)   # head-pairs 3-5, needed ~mid-attention

                    # ---- gathered K back to SBUF: kT_all[f] [128, 1024] ----
                    kT_all = [wpool.tile([128, T], BF16, tag=f"kta{i}",
                                         name=f"kta{i}") for i in range(6)]
                    for f in range(6):
                        for s in range(2):
                            dst = kT_all[f].rearrange(
                                "p (t two c) -> p t two c", two=2, c=128)[:, :, s, :]
                            nc.sync.dma_start(
                                dst, kb_out[f // 3][s, f % 3].rearrange(
                                    "p (t c) -> p t c", c=128))
                    # ---- gathered V back: v_all[g] [128, 12, 65] ----
                    # local tile t of side s = global tile 2t+s; halves h by t//2
                    v_all = [wpool.tile([128, 12, 65], BF16, tag=f"vall{i}",
                                        name=f"vall{i}") for i in range(NTT)]
                    for g in range(NTT):
                        s, t = g % 2, g // 2
                        nc.sync.dma_start(
                            v_all[g],
                            vb_out[t // 2][s, t % 2].rearrange("p (h d) -> p h d", d=65))

                # ---------- attention: 12 (pair, half) units, pipelined ----
                attoT = [lnpool.tile([128, TC], BF16, tag=f"attoT{i}", name=f"attoT{i}")
                         for i in range(NKT)]
                # prefetch fc weights during attention (layer-long pool)
                fcw_sb = lnpool.tile([128, NKT, 4 * D], BF16, name="fcw_sb")
                nc.sync.dma_start(fcw_sb,
                                  fcw_d[l].rearrange("(t p) f -> p t f", p=128))
                with tc.tile_pool(name=f"sps{l}", bufs=2, space="PSUM") as sps, \
                     tc.tile_pool(name=f"ops{l}", bufs=2, space="PSUM") as ops, \
                     tc.tile_pool(name=f"tps{l}", bufs=2, space="PSUM") as tps, \
                     tc.tile_pool(name=f"attp{l}", bufs=3) as attp:

                    att_tiles = [None] * 12
                    psT_tiles = [None] * 6

                    def qk_unit(u):
                        pr, hh = u // 2, u % 2
                        hs = slice(hh * 64, hh * 64 + 64)
                        # attT [128, 8 kt, 512 q]
                        attT = attp.tile([128, NTT, TC], BF16, tag="attT",
                                         name="attT")
                        att_tiles[u] = attT
                        for k2 in range(4):          # kt pairs (2k2, 2k2+1)
                            c0 = k2 * 128            # q-col start (slot k2)
                            w = TC - c0
                            ps2 = sps.tile([128, 2, TC], F32, tag="sps", name="sps")
                            for i in range(2):
                                kt = 2 * k2 + i
                                nc.tensor.matmul(
                                    ps2[:, i, 0:w],
                                    kT_all[pr][hs, kt * 128:(kt + 1) * 128],
                                    qk_sb[pr][hs, c0:TC],
                                    start=True, stop=True)
                            nc.scalar.activation(
                                attT[:, 2 * k2:2 * k2 + 2, c0:TC],
                                ps2[:, :, 0:w], AF.Exp, scale=0.125)
                        # gpsimd is busy with the K/V collectives early on
                        meng = nc.vector if u < 4 else nc.gpsimd
                        for j in range(NSL):
                            # mask the (2j, 2j+1) key pair for query slot j
                            meng.tensor_mul(
                                attT[:, 2 * j:2 * j + 2, j * 128:(j + 1) * 128],
                                attT[:, 2 * j:2 * j + 2, j * 128:(j + 1) * 128],
                                md_sb)

                    def av_unit(u):
                        pr, hh = u // 2, u % 2
                        h = 2 * pr + hh
                        attT = att_tiles[u]
                        if hh == 0:
                            psT_tiles[pr] = tps.tile([128, TC], BF16, tag="psT",
                                                     name="psT")
                        psT = psT_tiles[pr]
                        for j in range(NSL):
                            po = ops.tile([128, 65], F32, tag="po", name="po")
                            for kt in range(2 * j + 2):
                                nc.tensor.matmul(
                                    po, attT[:, kt, j * 128:(j + 1) * 128],
                                    v_all[kt][:, h, :],
                                    start=(kt == 0), stop=(kt == 2 * j + 1))
                            r_sb = scratch.tile([128, 1], F32, tag="r_sb", name="r_sb")
                            ao = scratch.tile([128, 64], BF16, tag="ao", name="ao")
                            nc.vector.reciprocal(r_sb, po[:, 64:65])
                            nc.vector.tensor_scalar_mul(ao, po[:, 0:64], r_sb)
                            nc.tensor.transpose(
                                psT[hh * 64:hh * 64 + 64,
                                    j * 128:(j + 1) * 128],
                                ao, ident,
                                tile_position=(0, hh * 64))
                        if hh == 1:
                            nc.vector.tensor_copy(attoT[pr], psT)

                    qk_unit(0)
                    qk_unit(1)
                    for u in range(12):
                        if u + 2 < 12:
                            qk_unit(u + 2)
                        av_unit(u)

                # ---------- proj + residual (LN2 stats fused, lag 1) ----------
                with tc.tile_pool(name=f"pps{l}", bufs=4, space="PSUM") as pps, \
                     tc.tile_pool(name=f"stps{l}b", bufs=1, space="PSUM") as stps_b:
                    s12 = _ln_stats_open(stps_b)
                    for ot in range(NKT):
                        ps = pps.tile([128, TC], F32, tag="pps", name="pps")
                        for kt in range(NKT):
                            nc.tensor.matmul(
                                ps, pw_sb[kt][:, ot * 128:(ot + 1) * 128],
                                attoT[kt],
                                start=(kt == 0), stop=(kt == NKT - 1))
                        if ot % 2 == 0:
                            nc.vector.scalar_tensor_tensor(
                                xt[ot], ps, pb_sb[:, ot:ot + 1],
                                xt[ot], op0=ALU.add, op1=ALU.add)
                        else:
                            tmp = scratch.tile([128, TC], F32, tag="rtmp",
                                               name="rtmp")
                            nc.scalar.activation(tmp, ps, AF.Identity,
                                                 bias=pb_sb[:, ot:ot + 1])
                            nc.gpsimd.tensor_add(xt[ot], xt[ot], tmp)
                        if ot >= 1:
                            _ln_stats_accum(nc, s12, ot - 1, xt[ot - 1], scratch)
                    _ln_stats_accum(nc, s12, NKT - 1, xt[NKT - 1], scratch)
                    m2, r2 = _ln_stats_finish(nc, s12, small, scratch)
                actx.close()

                # ---------- LN2 + MLP (next-LN stats fused into fc2) ------
                h2in = [lnpool.tile([128, TC], BF16, tag=f"hbf{i}", name=f"hbf{i}")
                        for i in range(NKT)]

                with tc.tile_pool(name=f"stps{l}c", bufs=1, space="PSUM") as stps_c, \
                     tc.tile_pool(name=f"mlpps{l}", bufs=3, space="PSUM") as mlpps, \
                     tc.tile_pool(name=f"mlpw{l}", bufs=1) as mlpw, \
                     tc.tile_pool(name=f"h2p{l}", bufs=1) as h2p:
                    fc2w_sb = mlpw.tile([128, 24, D], BF16, name="fc2w_sb")
                    nc.sync.dma_start(fc2w_sb,
                                      fc2w_d[l].rearrange("(t p) f -> p t f", p=128))
                    _ln_apply(tc, nc, xt, m2, r2, ln_sb[2], ln_sb[3], h2in, scratch)

                    h2c = [h2p.tile([128, TC], BF16, tag=f"h2c{f}", name=f"h2c{f}")
                           for f in range(24)]
                    for f in range(24):
                        ps = mlpps.tile([128, TC], F32, tag="fcps", name="fcps")
                        for kt in range(NKT):
                            nc.tensor.matmul(
                                ps, fcw_sb[:, kt, f * 128:(f + 1) * 128],
                                h2in[kt],
                                start=(kt == 0), stop=(kt == NKT - 1))
                        nc.scalar.activation(h2c[f], ps, AF.Gelu_apprx_tanh,
                                             bias=fcb_sb[:, f:f + 1])
                    s12 = _ln_stats_open(stps_c)
                    for ot in range(NKT):
                        ps = mlpps.tile([128, TC], F32, tag="fc2ps", name="fc2ps")
                        for kt in range(24):
                            nc.tensor.matmul(ps, fc2w_sb[:, kt, ot * 128:(ot + 1) * 128],
                                             h2c[kt],
                                             start=(kt == 0), stop=(kt == 23))
                        if ot % 2 == 0:
                            nc.vector.scalar_tensor_tensor(
                                xt[ot], ps, fc2b_sb[:, ot:ot + 1],
                                xt[ot], op0=ALU.add, op1=ALU.add)
                        else:
                            tmp = scratch.tile([128, TC], F32, tag="rtmp",
                                               name="rtmp")
                            nc.scalar.activation(tmp, ps, AF.Identity,
                                                 bias=fc2b_sb[:, ot:ot + 1])
                            nc.gpsimd.tensor_add(xt[ot], xt[ot], tmp)
                        if ot >= 1:
                            _ln_stats_accum(nc, s12, ot - 1, xt[ot - 1], scratch)
                    _ln_stats_accum(nc, s12, NKT - 1, xt[NKT - 1], scratch)
                    pending_stats = _ln_stats_finish(nc, s12, small, scratch)

        # ---------- final LN + xf AllGather + lm_head ----------
        with ExitStack() as fctx:
            lnpool = fctx.enter_context(tc.tile_pool(name="lnfp", bufs=1))
            biasp = fctx.enter_context(tc.tile_pool(name="biasf", bufs=1))
            scratch = fctx.enter_context(tc.tile_pool(name="scrf", bufs=3))
            lmwp = fctx.enter_context(tc.tile_pool(name="lmw", bufs=4))

            lnfg_sb = biasp.tile([128, 6], F32)
            nc.sync.dma_start(lnfg_sb, lnf_d[0].rearrange("(t p) -> p t", p=128))
            lnfb_sb = biasp.tile([128, 6], F32)
            nc.sync.dma_start(lnfb_sb, lnf_d[1].rearrange("(t p) -> p t", p=128))

            lm_wt = {}
            def lm_fetch(vc):
                wt = lmwp.tile([128, NKT, 512], BF16, tag="lmw_t", name="lmw_t")
                nc.sync.dma_start(
                    wt, lmw_d[:, vc * 512:(vc + 1) * 512]
                    .rearrange("(t p) v -> p t v", p=128))
                lm_wt[vc] = wt
            lm_fetch(0)
            lm_fetch(1)

            xf_own = [lnpool.tile([128, TC], BF16, tag=f"xo{i}", name=f"xo{i}")
                      for i in range(NKT)]
            m0, r0 = pending_stats
            _ln_apply(tc, nc, xt, m0, r0, lnfg_sb, lnfb_sb, xf_own, scratch)

            xb_in = [dram.tile([3, 128, TC], BF16, tag=f"xbi{h}", name=f"xbi{h}")
                     for h in range(2)]
            xb_out = [dram.tile([2, 3, 128, TC], BF16, tag=f"xbo{h}", name=f"xbo{h}")
                      for h in range(2)]
            xf_all = [lnpool.tile([128, T], BF16, tag=f"xa{i}", name=f"xa{i}")
                      for i in range(NKT)]
            for h in range(2):
                for i in range(3):
                    nc.gpsimd.dma_start(xb_in[h][i], xf_own[3 * h + i])
                nc.gpsimd.collective_compute(
                    "AllGather", mybir.AluOpType.bypass,
                    replica_groups=GROUPS,
                    ins=[xb_in[h][:].opt()], outs=[xb_out[h][:].opt()])
                for i in range(3):
                    f = 3 * h + i
                    for s in range(2):
                        dst = xf_all[f].rearrange(
                            "p (t two c) -> p t two c", two=2, c=128)[:, :, s, :]
                        nc.sync.dma_start(
                            dst, xb_out[h][s, i].rearrange("p (t c) -> p t c", c=128))

            with tc.tile_pool(name="lmps", bufs=4, space="PSUM") as lmps, \
                 tc.tile_pool(name="lmev", bufs=6) as lmev:
                for vc in range(NVC):
                    if vc + 2 < NVC:
                        lm_fetch(vc + 2)
                    wt = lm_wt.pop(vc)
                    for tt in range(NTT):
                        ps = lmps.tile([128, 512], F32, tag="lmps", name="lmps")
                        for kt in range(NKT):
                            nc.tensor.matmul(
                                ps, xf_all[kt][:, tt * 128:(tt + 1) * 128],
                                wt[:, kt, :],
                                start=(kt == 0), stop=(kt == NKT - 1))
                        ev = lmev.tile([128, 512], F16, tag="lmev", name="lmev")
                        if tt % 2 == 0:
                            nc.scalar.copy(ev, ps)
                        else:
                            nc.vector.tensor_copy(ev, ps)
                        nc.sync.dma_start(
                            out_d[tt * 128:(tt + 1) * 128,
                                  vc * 512:(vc + 1) * 512], ev)
    nc.finalize()
    return nc


_NC_CACHE = None


def _get_nc():
    global _NC_CACHE
    if _NC_CACHE is None:
        _NC_CACHE = build_bass()
    return _NC_CACHE


def make_in_maps(idx, layer_num, wte, wpe, ln1_g, ln1_b, attn_w, attn_b, proj_w,
                 proj_b, ln2_g, ln2_b, fc_w, fc_b, fc2_w, fc2_b, lnf_g, lnf_b, lm_w):
    bf = ml_dtypes.bfloat16
    idx = np.asarray(idx)
    f32 = np.float32
    wte = np.asarray(wte, f32)
    wpe = np.asarray(wpe, f32)
    x0 = wte[idx] + wpe[:T]                      # [B,T,D] fp32 host embedding

    attn_w = np.asarray(attn_w, f32)
    attn_b = np.asarray(attn_b, f32)
    proj_w = np.asarray(proj_w, f32)
    qkw = np.ascontiguousarray(attn_w[:, :, :2 * D]).astype(bf)
    vw = np.ascontiguousarray(attn_w[:, :, 2 * D:]).astype(bf)
    pw = proj_w.astype(bf)
    fcw = np.asarray(fc_w, f32).astype(bf)
    fc2w = np.asarray(fc2_w, f32).astype(bf)
    qkb = np.ascontiguousarray(attn_b[:, :2 * D])
    vb = np.ascontiguousarray(attn_b[:, 2 * D:])            # [L, D]
    pb_fold = np.einsum('ld,lde->le', vb, proj_w) + np.asarray(proj_b, f32)
    lnp = np.stack([np.asarray(ln1_g, f32), np.asarray(ln1_b, f32),
                    np.asarray(ln2_g, f32), np.asarray(ln2_b, f32)], axis=1)
    lnf = np.stack([np.asarray(lnf_g, f32), np.asarray(lnf_b, f32)], axis=0)

    lmw_pad = np.zeros((D, VPAD), f32)
    lmw_pad[:, :V] = np.asarray(lm_w, f32)
    lmw_bf = lmw_pad.astype(bf)

    tril = (np.arange(128)[:, None] <= np.arange(128)[None, :]).astype(np.float32)
    md = np.zeros((2, 128, 2, 128), np.float32)
    md[0, :, 0, :] = tril            # side 0: diag tile is its own slot tile
    md[0, :, 1, :] = 0.0             # side 0: extra odd tile fully masked
    md[1, :, 0, :] = 1.0             # side 1: even tile fully visible
    md[1, :, 1, :] = tril            # side 1: diag on the odd tile
    md = md.astype(bf)

    tok_idx = [np.concatenate([np.arange(128) + 128 * (2 * t + s)
                               for t in range(4)]) for s in range(2)]

    in_maps = []
    for core in range(8):
        b = core // 2
        s = core % 2
        vs = s * VSH
        in_maps.append(dict(
            xT=np.ascontiguousarray(x0[b][tok_idx[s]].T),
            qkw=qkw, vw=vw, pw=pw, fcw=fcw, fc2w=fc2w,
            qkb=qkb, pb=pb_fold,
            fcb=np.asarray(fc_b, f32), fc2b=np.asarray(fc2_b, f32),
            lnp=lnp, lnf=lnf, md=md[s],
            lmw=np.ascontiguousarray(lmw_bf[:, vs:vs + VSH]),
        ))
    return in_maps


def kernel(**inputs):
    global LAST_RESULT
    in_maps = make_in_maps(**inputs)
    nc = _get_nc()
    res = run_bass_kernel_spmd(nc, in_maps, core_ids=list(range(8)), trace=TRACE)
    LAST_RESULT = res

    logits = np.empty((B, T, V), np.float32)
    for b in range(B):
        lo = res.results[2 * b]["out"].astype(np.float32)
        hi = res.results[2 * b + 1]["out"].astype(np.float32)
        logits[b, :, :VSH] = lo
        logits[b, :, VSH:] = hi[:, :V - VSH]
    return logits


# revision 42
# speedup vs baseline: 1.3787x; 1.0301x over previous
"""GPT-2-ish forward (B=4, T=1024, D=768, H=12, L=2, V=50257) on 8 trn2 cores.

Sharding (v3): core pair (2b, 2b+1) sequence-splits the trunk for batch b.
Side s = core%2 owns the even (s=0) or odd (s=1) 128-token tiles of the
1024-token sequence: tiles {s, s+2, s+4, s+6} (interleaving balances the
causal attention load: key-tile needs sum to 16 vs 20). Each core runs
LN / qkv / proj / MLP on its own 512 tokens; K and V are pair-AllGathered
per layer (DRAM bounce) so attention sees all 1024 keys; the final-LN
output is pair-AllGathered before a vocab-split lm_head identical to v2
(each core: all 1024 tokens x 25600 vocab columns).

The SPMD program is identical on every core. Side-dependent causal
masking is data: slot j's last two key tiles (2j, 2j+1) are multiplied by
a per-core [128, 2, 128] mask md = (tril, zeros) on side 0 and
(ones, tril) on side 1. QK scores/exps only cover key tile kt from query
slot kt//2 onward; exp activations are issued per kt-pair on a
[128, 2, 512] PSUM tile to halve the activation-instruction count.

Engine layout as v2: PE ramps to 2.4 GHz only when streaming, so QK
matmuls of attention unit u+1 are issued before att@V of unit u; PSUM
evictions alternate DVE/scalar; GPSIMD (Pool, SBUF-only) takes masks,
LN subtract and residual adds. All matmuls bf16 with fp32 PSUM
accumulation; residual fp32; logits f16, upcast on host.
"""

import numpy as np
import ml_dtypes
from contextlib import ExitStack

import concourse.bass as bass
from concourse import bacc
import concourse.mybir as mybir
import concourse.tile as tile
from concourse.bass_utils import run_bass_kernel_spmd
from concourse.masks import make_identity

BF16 = mybir.dt.bfloat16
F32 = mybir.dt.float32
F16 = mybir.dt.float16
AF = mybir.ActivationFunctionType
ALU = mybir.AluOpType

V = 50257
VPAD = 51200          # 2 * 25600
VSH = VPAD // 2       # per-core vocab shard
D = 768
H = 12
HD = 64
L = 2
T = 1024
TC = 512              # tokens per core (trunk)
B = 4
EPS = 1e-5
NKT = D // 128        # 6 k-tiles over D
NTT = T // 128        # 8 global token-tiles
NSL = TC // 128       # 4 local token slots
NVC = VSH // 512      # 50 lm vocab chunks per core
GROUPS = [[0, 1], [2, 3], [4, 5], [6, 7]]

TRACE = False
LAST_RESULT = None

_SINGLES = {}


def _ln_stats_open(stats_ps):
    s1 = stats_ps.tile([128, TC], F32, tag="s1", name="s1")
    s2 = stats_ps.tile([128, TC], F32, tag="s2", name="s2")
    return s1, s2


def _ln_stats_accum(nc, s12, kt, x_tile, scratch):
    """Accumulate sum / sum-of-squares of one feature tile into PSUM.
    Emitted right after the residual update of that tile so the LN of the
    next sublayer has no serial stats phase."""
    s1, s2 = s12
    onesq = _SINGLES["onesq"]
    xbf = scratch.tile([128, TC], BF16, tag="xbf", name="xbf")
    sq = scratch.tile([128, TC], BF16, tag="sq", name="sq")
    nc.gpsimd.tensor_copy(xbf, x_tile)
    nc.vector.tensor_mul(sq, x_tile, x_tile)
    nc.tensor.matmul(s1, onesq, xbf, start=(kt == 0), stop=(kt == NKT - 1))
    nc.tensor.matmul(s2, onesq, sq, start=(kt == 0), stop=(kt == NKT - 1))


def _ln_stats_finish(nc, s12, small, scratch):
    s1, s2 = s12
    eps128 = _SINGLES["eps128"]
    mean = small.tile([128, TC], F32, tag="mean", name="mean")
    rstd = small.tile([128, TC], F32, tag="rstd", name="rstd")
    var = scratch.tile([128, TC], F32, tag="var", name="var")
    nc.vector.tensor_scalar_mul(mean, s1, 1.0 / D)
    nc.vector.tensor_mul(var, mean, mean)
    nc.vector.scalar_tensor_tensor(var, s2, 1.0 / D, var,
                                   op0=ALU.mult, op1=ALU.subtract)
    nc.scalar.activation(var, var, AF.Sqrt, bias=eps128)
    nc.vector.reciprocal(rstd, var)
    return mean, rstd


def _ln_stats(tc, nc, xt, small, scratch, stats_ps):
    """Standalone stats (used when there is no preceding loop to fuse into)."""
    s12 = _ln_stats_open(stats_ps)
    for kt in range(NKT):
        _ln_stats_accum(nc, s12, kt, xt[kt], scratch)
    return _ln_stats_finish(nc, s12, small, scratch)


def _ln_apply(tc, nc, xt, mean, rstd, g_sb, b_sb, out_tiles, scratch):
    for kt in range(NKT):
        t1 = scratch.tile([128, TC], F32, tag="lnt1", name="lnt1")
        eng = nc.gpsimd if kt % 2 == 0 else nc.vector
        eng.tensor_sub(t1, xt[kt], mean)
        nc.vector.scalar_tensor_tensor(t1, t1, g_sb[:, kt:kt + 1], rstd,
                                       op0=ALU.mult, op1=ALU.mult)
        nc.scalar.activation(out_tiles[kt], t1, AF.Identity,
                             bias=b_sb[:, kt:kt + 1])


def build_bass():
    nc = bacc.Bacc(None, target_bir_lowering=False)
    # ---- DRAM I/O (per-core shard views) ----
    xT_d = nc.dram_tensor("xT", [D, TC], F32, kind="ExternalInput")
    qkw_d = nc.dram_tensor("qkw", [L, D, 2 * D], BF16, kind="ExternalInput")
    vw_d = nc.dram_tensor("vw", [L, D, D], BF16, kind="ExternalInput")
    pw_d = nc.dram_tensor("pw", [L, D, D], BF16, kind="ExternalInput")
    fcw_d = nc.dram_tensor("fcw", [L, D, 4 * D], BF16, kind="ExternalInput")
    fc2w_d = nc.dram_tensor("fc2w", [L, 4 * D, D], BF16, kind="ExternalInput")
    qkb_d = nc.dram_tensor("qkb", [L, 2 * D], F32, kind="ExternalInput")
    pb_d = nc.dram_tensor("pb", [L, D], F32, kind="ExternalInput")
    fcb_d = nc.dram_tensor("fcb", [L, 4 * D], F32, kind="ExternalInput")
    fc2b_d = nc.dram_tensor("fc2b", [L, D], F32, kind="ExternalInput")
    ln_d = nc.dram_tensor("lnp", [L, 4, D], F32, kind="ExternalInput")
    lnf_d = nc.dram_tensor("lnf", [2, D], F32, kind="ExternalInput")
    md_d = nc.dram_tensor("md", [128, 2, 128], BF16, kind="ExternalInput")
    lmw_d = nc.dram_tensor("lmw", [D, VPAD], BF16, kind="ExternalInput")
    out_d = nc.dram_tensor("out", [TC, VPAD], F16, kind="ExternalOutput")

    with tile.TileContext(nc) as tc, ExitStack() as octx:
        singles = octx.enter_context(tc.tile_pool(name="singles", bufs=1))
        resid = octx.enter_context(tc.tile_pool(name="resid", bufs=1))
        dram = octx.enter_context(tc.tile_pool(name="dram", bufs=1, space="DRAM"))

        onesq = singles.tile([128, 128], BF16)
        nc.vector.memset(onesq, 1.0)
        eps128 = singles.tile([128, 1], F32)
        nc.vector.memset(eps128, EPS)
        ident = singles.tile([128, 128], BF16)
        make_identity(nc, ident)
        _SINGLES["onesq"] = onesq
        _SINGLES["eps128"] = eps128

        md_sb = singles.tile([128, 2, 128], BF16)
        nc.sync.dma_start(md_sb, md_d[:, :, :])

        small = octx.enter_context(tc.tile_pool(name="small", bufs=2))

        xt = [resid.tile([128, TC], F32, tag=f"xt{i}", name=f"xt{i}")
              for i in range(NKT)]
        for kt in range(NKT):
            nc.sync.dma_start(xt[kt], xT_d[kt * 128:(kt + 1) * 128, :])

        pending_stats = None   # (mean, rstd) for the next LN, fused into fc2

        for l in range(L):
            with ExitStack() as lctx:
                lnpool = lctx.enter_context(tc.tile_pool(name=f"ln{l}", bufs=1))
                biasp = lctx.enter_context(tc.tile_pool(name=f"bias{l}", bufs=1))
                scratch = lctx.enter_context(tc.tile_pool(name=f"scr{l}", bufs=3))
                actx = ExitStack()
                wpool = actx.enter_context(tc.tile_pool(name=f"w{l}", bufs=1))

                qkw_sb = wpool.tile([128, NKT, 2 * D], BF16, name="qkw_sb")
                nc.sync.dma_start(qkw_sb,
                                  qkw_d[l].rearrange("(t p) f -> p t f", p=128))
                vw_sb = [wpool.tile([128, D], BF16, tag=f"vw{i}", name=f"vw{i}")
                         for i in range(NKT)]
                for kt in range(NKT):
                    nc.sync.dma_start(vw_sb[kt], vw_d[l][kt * 128:(kt + 1) * 128, :])
                pw_sb = [wpool.tile([128, D], BF16, tag=f"pw{i}", name=f"pw{i}")
                         for i in range(NKT)]
                for kt in range(NKT):
                    nc.sync.dma_start(pw_sb[kt], pw_d[l][kt * 128:(kt + 1) * 128, :])

                qkb_sb = biasp.tile([128, 12], F32)
                nc.sync.dma_start(qkb_sb, qkb_d[l].rearrange("(t p) -> p t", p=128))
                pb_sb = biasp.tile([128, 6], F32)
                nc.sync.dma_start(pb_sb, pb_d[l].rearrange("(t p) -> p t", p=128))
                fcb_sb = biasp.tile([128, 24], F32)
                nc.sync.dma_start(fcb_sb, fcb_d[l].rearrange("(t p) -> p t", p=128))
                fc2b_sb = biasp.tile([128, 6], F32)
                nc.sync.dma_start(fc2b_sb, fc2b_d[l].rearrange("(t p) -> p t", p=128))
                ln_sb = []
                for i in range(4):
                    t = biasp.tile([128, 6], F32, tag=f"lnp{i}", name=f"lnp{i}")
                    nc.sync.dma_start(t, ln_d[l][i].rearrange("(t p) -> p t", p=128))
                    ln_sb.append(t)

                # ---------- LN1 ----------
                h_bf = [lnpool.tile([128, TC], BF16, tag=f"hbf{i}", name=f"hbf{i}")
                        for i in range(NKT)]
                qk_sb = [wpool.tile([128, TC], BF16, tag=f"qk{i}", name=f"qk{i}")
                         for i in range(12)]

                kb_in = [dram.tile([3, 128, TC], BF16,
                                   tag=f"kbi{l}{h}", name=f"kbi{l}{h}")
                         for h in range(2)]
                kb_out = [dram.tile([2, 3, 128, TC], BF16,
                                    tag=f"kbo{l}{h}", name=f"kbo{l}{h}")
                          for h in range(2)]

                def k_gather(half):
                    for i in range(3):
                        nc.gpsimd.dma_start(kb_in[half][i],
                                            qk_sb[6 + 3 * half + i])
                    nc.gpsimd.collective_compute(
                        "AllGather", mybir.AluOpType.bypass,
                        replica_groups=GROUPS,
                        ins=[kb_in[half][:].opt()],
                        outs=[kb_out[half][:].opt()])

                with tc.tile_pool(name=f"stps{l}a", bufs=1, space="PSUM") as stats_ps, \
                     tc.tile_pool(name=f"qkps{l}", bufs=3, space="PSUM") as qkps:
                    if pending_stats is None:
                        m0, r0 = _ln_stats(tc, nc, xt, small, scratch, stats_ps)
                    else:
                        m0, r0 = pending_stats
                    _ln_apply(tc, nc, xt, m0, r0, ln_sb[0], ln_sb[1], h_bf, scratch)

                    # K features first (f 6..11) so the K gather starts early
                    for f in list(range(6, 12)) + list(range(6)):
                        ps = qkps.tile([128, TC], F32, tag="qkps", name="qkps")
                        for kt in range(NKT):
                            nc.tensor.matmul(
                                ps, qkw_sb[:, kt, f * 128:(f + 1) * 128],
                                h_bf[kt],
                                start=(kt == 0), stop=(kt == NKT - 1))
                        nc.scalar.activation(qk_sb[f], ps, AF.Identity,
                                             bias=qkb_sb[:, f:f + 1])
                        if f == 8:
                            k_gather(0)   # covers head-pairs 0-2

                    # ---------- V own tiles [128, 12, 65] (incl ones col) ----
                    # V gather halves are emitted mid-loop: half h covers
                    # global key tiles 4h..4h+3, needed by att@V slots 2h..
                    vb_in = [dram.tile([2, 128, 12 * 65], BF16, tag=f"vbi{l}{h}",
                                       name=f"vbi{l}{h}") for h in range(2)]
                    vb_out = [dram.tile([2, 2, 128, 12 * 65], BF16, tag=f"vbo{l}{h}",
                                        name=f"vbo{l}{h}") for h in range(2)]
                    v_own = [wpool.tile([128, 12, 65], BF16, tag=f"vown{i}",
                                        name=f"vown{i}") for i in range(NSL)]
                    for tt in range(NSL):
                        nc.vector.memset(v_own[tt][:, :, 64:65], 1.0)
                        for vc in range(2):
                            vs = slice(vc * 384, (vc + 1) * 384)
                            ps = qkps.tile([128, 384], F32, tag="vps", name="vps")
                            for kt in range(NKT):
                                nc.tensor.matmul(
                                    ps, h_bf[kt][:, tt * 128:(tt + 1) * 128],
                                    vw_sb[kt][:, vs],
                                    start=(kt == 0), stop=(kt == NKT - 1))
                            nc.vector.tensor_copy(
                                v_own[tt][:, vc * 6:(vc + 1) * 6, 0:64],
                                ps.rearrange("p (h d) -> p h d", d=64))
                        if tt % 2 == 1:
                            h = tt // 2
                            for i in range(2):
                                nc.gpsimd.dma_start(
                                    vb_in[h][i],
                                    v_own[2 * h + i].rearrange("p h d -> p (h d)"))
                            nc.gpsimd.collective_compute(
                                "AllGather", mybir.AluOpType.bypass,
                                replica_groups=GROUPS,
                                ins=[vb_in[h][:].opt()], outs=[vb_out[h][:].opt()])
                    k_gather(1)   # head-pairs 3-5, needed ~mid-attention

                    # ---- gathered K back to SBUF: kT_all[f] [128, 1024] ----
                    kT_all = [wpool.tile([128, T], BF16, tag=f"kta{i}",
                                         name=f"kta{i}") for i in range(6)]
                    for f in range(6):
                        for s in range(2):
                            dst = kT_all[f].rearrange(
                                "p (t two c) -> p t two c", two=2, c=128)[:, :, s, :]
                            nc.sync.dma_start(
                                dst, kb_out[f // 3][s, f % 3].rearrange(
                                    "p (t c) -> p t c", c=128))
                    # ---- gathered V back: v_all[g] [128, 12, 65] ----
                    # local tile t of side s = global tile 2t+s; halves h by t//2
                    v_all = [wpool.tile([128, 12, 65], BF16, tag=f"vall{i}",
                                        name=f"vall{i}") for i in range(NTT)]
                    for g in range(NTT):
                        s, t = g % 2, g // 2
                        nc.sync.dma_start(
                            v_all[g],
                            vb_out[t // 2][s, t % 2].rearrange("p (h d) -> p h d", d=65))

                # ---------- attention: 12 (pair, half) units, pipelined ----
                attoT = [lnpool.tile([128, TC], BF16, tag=f"attoT{i}", name=f"attoT{i}")
                         for i in range(NKT)]
                # prefetch fc weights during attention (layer-long pool)
                fcw_sb = lnpool.tile([128, NKT, 4 * D], BF16, name="fcw_sb")
                nc.sync.dma_start(fcw_sb,
                                  fcw_d[l].rearrange("(t p) f -> p t f", p=128))
                with tc.tile_pool(name=f"sps{l}", bufs=2, space="PSUM") as sps, \
                     tc.tile_pool(name=f"ops{l}", bufs=2, space="PSUM") as ops, \
                     tc.tile_pool(name=f"tps{l}", bufs=2, space="PSUM") as tps, \
                     tc.tile_pool(name=f"attp{l}", bufs=3) as attp:

                    att_tiles = [None] * 12
                    psT_tiles = [None] * 6

                    def qk_unit(u):
                        pr, hh = u // 2, u % 2
                        hs = slice(hh * 64, hh * 64 + 64)
                        # attT [128, 8 kt, 512 q]
                        attT = attp.tile([128, NTT, TC], BF16, tag="attT",
                                         name="attT")
                        att_tiles[u] = attT
                        for k2 in range(4):          # kt pairs (2k2, 2k2+1)
                            c0 = k2 * 128            # q-col start (slot k2)
                            w = TC - c0
                            ps2 = sps.tile([128, 2, TC], F32, tag="sps", name="sps")
                            for i in range(2):
                                kt = 2 * k2 + i
                                nc.tensor.matmul(
                                    ps2[:, i, 0:w],
                                    kT_all[pr][hs, kt * 128:(kt + 1) * 128],
                                    qk_sb[pr][hs, c0:TC],
                                    start=True, stop=True)
                            nc.scalar.activation(
                                attT[:, 2 * k2:2 * k2 + 2, c0:TC],
                                ps2[:, :, 0:w], AF.Exp, scale=0.125)
                        # gpsimd is busy with the K/V collectives early on
                        meng = nc.vector if u < 4 else nc.gpsimd
                        for j in range(NSL):
                            # mask the (2j, 2j+1) key pair for query slot j
                            meng.tensor_mul(
                                attT[:, 2 * j:2 * j + 2, j * 128:(j + 1) * 128],
                                attT[:, 2 * j:2 * j + 2, j * 128:(j + 1) * 128],
                                md_sb)

                    def av_unit(u):
                        pr, hh = u // 2, u % 2
                        h = 2 * pr + hh
                        attT = att_tiles[u]
                        if hh == 0:
                            psT_tiles[pr] = tps.tile([128, TC], BF16, tag="psT",
                                                     name="psT")
                        psT = psT_tiles[pr]
                        for j in range(NSL):
                            po = ops.tile([128, 65], F32, tag="po", name="po")
                            for kt in range(2 * j + 2):
                                nc.tensor.matmul(
                                    po, attT[:, kt, j * 128:(j + 1) * 128],
                                    v_all[kt][:, h, :],
                                    start=(kt == 0), stop=(kt == 2 * j + 1))
                            r_sb = scratch.tile([128, 1], F32, tag="r_sb", name="r_sb")
                            ao = scratch.tile([128, 64], BF16, tag="ao", name="ao")
                            nc.vector.reciprocal(r_sb, po[:, 64:65])
                            nc.vector.tensor_scalar_mul(ao, po[:, 0:64], r_sb)
                            nc.tensor.transpose(
                                psT[hh * 64:hh * 64 + 64,
                                    j * 128:(j + 1) * 128],
                                ao, ident,
                                tile_position=(0, hh * 64))
                        if hh == 1:
                            nc.vector.tensor_copy(attoT[pr], psT)

                    qk_unit(0)
                    qk_unit(1)
                    for u in range(12):
                        if u + 2 < 12:
                            qk_unit(u + 2)
                        av_unit(u)

                # ---------- proj + residual (LN2 stats fused, lag 1) ----------
                with tc.tile_pool(name=f"pps{l}", bufs=4, space="PSUM") as pps, \
                     tc.tile_pool(name=f"stps{l}b", bufs=1, space="PSUM") as stps_b:
                    s12 = _ln_stats_open(stps_b)
                    for ot in range(NKT):
                        ps = pps.tile([128, TC], F32, tag="pps", name="pps")
                        for kt in range(NKT):
                            nc.tensor.matmul(
                                ps, pw_sb[kt][:, ot * 128:(ot + 1) * 128],
                                attoT[kt],
                                start=(kt == 0), stop=(kt == NKT - 1))
                        if ot % 2 == 0:
                            nc.vector.scalar_tensor_tensor(
                                xt[ot], ps, pb_sb[:, ot:ot + 1],
                                xt[ot], op0=ALU.add, op1=ALU.add)
                        else:
                            tmp = scratch.tile([128, TC], F32, tag="rtmp",
                                               name="rtmp")
                            nc.scalar.activation(tmp, ps, AF.Identity,
                                                 bias=pb_sb[:, ot:ot + 1])
                            nc.gpsimd.tensor_add(xt[ot], xt[ot], tmp)
                        if ot >= 1:
                            _ln_stats_accum(nc, s12, ot - 1, xt[ot - 1], scratch)
                    _ln_stats_accum(nc, s12, NKT - 1, xt[NKT - 1], scratch)
                    m2, r2 = _ln_stats_finish(nc, s12, small, scratch)
                actx.close()

                # ---------- LN2 + MLP (next-LN stats fused into fc2) ------
                h2in = [lnpool.tile([128, TC], BF16, tag=f"hbf{i}", name=f"hbf{i}")
                        for i in range(NKT)]

                with tc.tile_pool(name=f"stps{l}c", bufs=1, space="PSUM") as stps_c, \
                     tc.tile_pool(name=f"mlpps{l}", bufs=3, space="PSUM") as mlpps, \
                     tc.tile_pool(name=f"mlpw{l}", bufs=1) as mlpw, \
                     tc.tile_pool(name=f"h2p{l}", bufs=1) as h2p:
                    fc2w_sb = mlpw.tile([128, 24, D], BF16, name="fc2w_sb")
                    nc.sync.dma_start(fc2w_sb,
                                      fc2w_d[l].rearrange("(t p) f -> p t f", p=128))
                    _ln_apply(tc, nc, xt, m2, r2, ln_sb[2], ln_sb[3], h2in, scratch)

                    h2c = [h2p.tile([128, TC], BF16, tag=f"h2c{f}", name=f"h2c{f}")
                           for f in range(24)]
                    for f in range(24):
                        ps = mlpps.tile([128, TC], F32, tag="fcps", name="fcps")
                        for kt in range(NKT):
                            nc.tensor.matmul(
                                ps, fcw_sb[:, kt, f * 128:(f + 1) * 128],
                                h2in[kt],
                                start=(kt == 0), stop=(kt == NKT - 1))
                        nc.scalar.activation(h2c[f], ps, AF.Gelu_apprx_tanh,
                                             bias=fcb_sb[:, f:f + 1])
                    s12 = _ln_stats_open(stps_c)
                    for ot in range(NKT):
                        ps = mlpps.tile([128, TC], F32, tag="fc2ps", name="fc2ps")
                        for kt in range(24):
                            nc.tensor.matmul(ps, fc2w_sb[:, kt, ot * 128:(ot + 1) * 128],
                                             h2c[kt],
                                             start=(kt == 0), stop=(kt == 23))
                        if ot % 2 == 0:
                            nc.vector.scalar_tensor_tensor(
                                xt[ot], ps, fc2b_sb[:, ot:ot + 1],
                                xt[ot], op0=ALU.add, op1=ALU.add)
                        else:
                            tmp = scratch.tile([128, TC], F32, tag="rtmp",
                                               name="rtmp")
                            nc.scalar.activation(tmp, ps, AF.Identity,
                                                 bias=fc2b_sb[:, ot:ot + 1])
                            nc.gpsimd.tensor_add(xt[ot], xt[ot], tmp)
                        if ot >= 1:
                            _ln_stats_accum(nc, s12, ot - 1, xt[ot - 1], scratch)
                    _ln_stats_accum(nc, s12, NKT - 1, xt[NKT - 1], scratch)
                    pending_stats = _ln_stats_finish(nc, s12, small, scratch)

        # ---------- final LN + xf AllGather + lm_head ----------
        with ExitStack() as fctx:
            lnpool = fctx.enter_context(tc.tile_pool(name="lnfp", bufs=1))
            biasp = fctx.enter_context(tc.tile_pool(name="biasf", bufs=1))
            scratch = fctx.enter_context(tc.tile_pool(name="scrf", bufs=3))
            lmwp = fctx.enter_context(tc.tile_pool(name="lmw", bufs=4))

            lnfg_sb = biasp.tile([128, 6], F32)
            nc.sync.dma_start(lnfg_sb, lnf_d[0].rearrange("(t p) -> p t", p=128))
            lnfb_sb = biasp.tile([128, 6], F32)
            nc.sync.dma_start(lnfb_sb, lnf_d[1].rearrange("(t p) -> p t", p=128))

            lm_wt = {}
            def lm_fetch(vc):
                wt = lmwp.tile([128, NKT, 512], BF16, tag="lmw_t", name="lmw_t")
                nc.sync.dma_start(
                    wt, lmw_d[:, vc * 512:(vc + 1) * 512]
                    .rearrange("(t p) v -> p t v", p=128))
                lm_wt[vc] = wt
            lm_fetch(0)
            lm_fetch(1)

            xf_own = [lnpool.tile([128, TC], BF16, tag=f"xo{i}", name=f"xo{i}")
                      for i in range(NKT)]
            m0, r0 = pending_stats
            _ln_apply(tc, nc, xt, m0, r0, lnfg_sb, lnfb_sb, xf_own, scratch)

            # lm_head: own 512 tokens x full padded vocab (no xf gather)
            with tc.tile_pool(name="lmps", bufs=4, space="PSUM") as lmps, \
                 tc.tile_pool(name="lmev", bufs=6) as lmev:
                for vc in range(VPAD // 512):
                    if vc + 2 < VPAD // 512:
                        lm_fetch(vc + 2)
                    wt = lm_wt.pop(vc)
                    for tt in range(NSL):
                        ps = lmps.tile([128, 512], F32, tag="lmps", name="lmps")
                        for kt in range(NKT):
                            nc.tensor.matmul(
                                ps, xf_own[kt][:, tt * 128:(tt + 1) * 128],
                                wt[:, kt, :],
                                start=(kt == 0), stop=(kt == NKT - 1))
                        ev = lmev.tile([128, 512], F16, tag="lmev", name="lmev")
                        if tt % 2 == 0:
                            nc.scalar.copy(ev, ps)
                        else:
                            nc.vector.tensor_copy(ev, ps)
                        nc.sync.dma_start(
                            out_d[tt * 128:(tt + 1) * 128,
                                  vc * 512:(vc + 1) * 512], ev)
    nc.finalize()
    return nc


_NC_CACHE = None


def _get_nc():
    global _NC_CACHE
    if _NC_CACHE is None:
        _NC_CACHE = build_bass()
    return _NC_CACHE


def make_in_maps(idx, layer_num, wte, wpe, ln1_g, ln1_b, attn_w, attn_b, proj_w,
                 proj_b, ln2_g, ln2_b, fc_w, fc_b, fc2_w, fc2_b, lnf_g, lnf_b, lm_w):
    bf = ml_dtypes.bfloat16
    idx = np.asarray(idx)
    f32 = np.float32
    wte = np.asarray(wte, f32)
    wpe = np.asarray(wpe, f32)
    x0 = wte[idx] + wpe[:T]                      # [B,T,D] fp32 host embedding

    attn_w = np.asarray(attn_w, f32)
    attn_b = np.asarray(attn_b, f32)
    proj_w = np.asarray(proj_w, f32)
    qkw = np.ascontiguousarray(attn_w[:, :, :2 * D]).astype(bf)
    vw = np.ascontiguousarray(attn_w[:, :, 2 * D:]).astype(bf)
    pw = proj_w.astype(bf)
    fcw = np.asarray(fc_w, f32).astype(bf)
    fc2w = np.asarray(fc2_w, f32).astype(bf)
    qkb = np.ascontiguousarray(attn_b[:, :2 * D])
    vb = np.ascontiguousarray(attn_b[:, 2 * D:])            # [L, D]
    pb_fold = np.einsum('ld,lde->le', vb, proj_w) + np.asarray(proj_b, f32)
    lnp = np.stack([np.asarray(ln1_g, f32), np.asarray(ln1_b, f32),
                    np.asarray(ln2_g, f32), np.asarray(ln2_b, f32)], axis=1)
    lnf = np.stack([np.asarray(lnf_g, f32), np.asarray(lnf_b, f32)], axis=0)

    lmw_pad = np.zeros((D, VPAD), f32)
    lmw_pad[:, :V] = np.asarray(lm_w, f32)
    lmw_bf = lmw_pad.astype(bf)

    tril = (np.arange(128)[:, None] <= np.arange(128)[None, :]).astype(np.float32)
    md = np.zeros((2, 128, 2, 128), np.float32)
    md[0, :, 0, :] = tril            # side 0: diag tile is its own slot tile
    md[0, :, 1, :] = 0.0             # side 0: extra odd tile fully masked
    md[1, :, 0, :] = 1.0             # side 1: even tile fully visible
    md[1, :, 1, :] = tril            # side 1: diag on the odd tile
    md = md.astype(bf)

    tok_idx = [np.concatenate([np.arange(128) + 128 * (2 * t + s)
                               for t in range(4)]) for s in range(2)]

    in_maps = []
    for core in range(8):
        b = core // 2
        s = core % 2
        in_maps.append(dict(
            xT=np.ascontiguousarray(x0[b][tok_idx[s]].T),
            qkw=qkw, vw=vw, pw=pw, fcw=fcw, fc2w=fc2w,
            qkb=qkb, pb=pb_fold,
            fcb=np.asarray(fc_b, f32), fc2b=np.asarray(fc2_b, f32),
            lnp=lnp, lnf=lnf, md=md[s],
            lmw=lmw_bf,
        ))
    return in_maps


def kernel(**inputs):
    global LAST_RESULT
    in_maps = make_in_maps(**inputs)
    nc = _get_nc()
    res = run_bass_kernel_spmd(nc, in_maps, core_ids=list(range(8)), trace=TRACE)
    LAST_RESULT = res

    tok_idx = [np.concatenate([np.arange(128) + 128 * (2 * t + s)
                               for t in range(4)]) for s in range(2)]
    logits = np.empty((B, T, V), np.float32)
    for b in range(B):
        for s in range(2):
            part = res.results[2 * b + s]["out"].astype(np.float32)
            logits[b, tok_idx[s]] = part[:, :V]
    return logits
